# revision 10
# baseline (speedup 1.0000x reference)
"""Self-contained Trainium2 Bass kernel for BertSelfAttention (relative_key_query).

kernel(**inputs) takes FULL unsharded inputs (as in setup_inputs()) and returns
the FULL (8, 1024, 1024) float32 output. Internally: data-parallel over the
batch dimension, one batch per NeuronCore across 8 cores, via
concourse run_bass_kernel_spmd.

v3: bf16 weights/activations (fp8 only for the relative-position window
tiles, whose small magnitude makes fp8 quantization negligible), DMA-transpose
for hs^T, resident WV, packed per-pair WQK streams, merged skew-read DMAs,
1-bank score tiles, and a software-pipelined emission order interleaving pair
P+1's projection/window chunks with pair P's score chunks so the window DRAM
round trip and the PSUM->fp8 copy drain hide behind score/ctx matmuls.
"""

import os
import numpy as np

import concourse.bacc as bacc
import concourse.mybir as mybir
import concourse.tile as tile

f32 = mybir.dt.float32
bf16 = mybir.dt.bfloat16
fp8 = mybir.dt.float8e4

S = 1024
D = 1024
H = 16
DH = 64
NT = 8
WIN = 1152
NPAIR = 8


def host_prep(hidden_states, attention_mask, W_qkv, b_qkv, dist_emb):
    import ml_dtypes

    B = hidden_states.shape[0]
    W = np.asarray(W_qkv, dtype=np.float32)
    b = np.asarray(b_qkv, dtype=np.float32)
    T = np.asarray(dist_emb, dtype=np.float32)

    # Per-pair packed QK weights: [pair][dchunk][128 d][256 cols(q128|k128)]
    qcols = np.zeros((8, 128), dtype=np.int64)
    kcols = np.zeros((8, 128), dtype=np.int64)
    for t in range(8):
        for j in range(128):
            h = 2 * t + (j >= 64)
            d = j % 64
            qcols[t, j] = h * 192 + d
            kcols[t, j] = h * 192 + 64 + d
    wqkp = np.zeros((8, 8, 128, 256), dtype=np.float32)
    for P in range(8):
        cols = np.concatenate([qcols[P], kcols[P]])
        wqkp[P] = W[:, cols].reshape(8, 128, 256)
    wqkp = np.ascontiguousarray(wqkp.reshape(8, 8 * 128 * 256)).astype(
        ml_dtypes.bfloat16)
    qk_idx = np.concatenate([qcols.reshape(-1), kcols.reshape(-1)])
    bQK = np.ascontiguousarray(b[qk_idx].reshape(16, 128).T)

    # Resident V weights: [128 d][dchunk*1024 + vcol(head-major)]
    vidx = np.array([h * 192 + 128 + d for h in range(H) for d in range(DH)])
    wvp = np.ascontiguousarray(
        W[:, vidx].reshape(8, 128, 1024).transpose(1, 0, 2).reshape(128, 8192)
    ).astype(ml_dtypes.bfloat16)
    bV = np.ascontiguousarray(b[vidx].reshape(1, 1024)).astype(
        ml_dtypes.bfloat16)

    T2 = np.zeros((128, 2048), dtype=np.float32)
    T2[0:64, 0:2047] = T.T
    T2[64:128, 0:2047] = T.T
    T2R = np.zeros((128, 2048), dtype=np.float32)
    T2R[0:64, 0:2047] = T.T[:, ::-1]
    T2R[64:128, 0:2047] = T.T[:, ::-1]
    T2 = T2.astype(ml_dtypes.bfloat16)
    T2R = T2R.astype(ml_dtypes.bfloat16)

    ones_r = np.ones((1, 128), dtype=np.float32).astype(ml_dtypes.bfloat16)
    id8_h = np.eye(128, dtype=np.float32).astype(ml_dtypes.float8_e4m3fn)
    idf_h = np.eye(128, dtype=np.float32)

    mask = np.asarray(attention_mask, dtype=np.float32).reshape(B, S)
    in_maps = []
    for bi in range(B):
        mhat = np.ascontiguousarray(mask[bi].reshape(8, 128).T)
        hsb = np.ascontiguousarray(
            np.asarray(hidden_states[bi], dtype=np.float32)
        ).astype(ml_dtypes.bfloat16)
        in_maps.append({
            "hs": hsb,
            "wqkp": wqkp, "bqk": bQK, "wv": wvp, "bv": bV,
            "t2": T2, "t2r": T2R, "ones_r": ones_r, "mhat": mhat,
            "id8_h": id8_h, "idf_h": idf_h,
        })
    return in_maps


def build_program(npair=NPAIR):
    nc = bacc.Bacc()
    hs_d = nc.declare_dram_parameter("hs", [S, D], bf16, isOutput=False)
    wqk_d = nc.declare_dram_parameter("wqkp", [8, 8 * 128 * 256], bf16,
                                      isOutput=False)
    bqk_d = nc.declare_dram_parameter("bqk", [128, 16], f32, isOutput=False)
    wv_d = nc.declare_dram_parameter("wv", [128, 8192], bf16, isOutput=False)
    bv_d = nc.declare_dram_parameter("bv", [1, 1024], bf16, isOutput=False)
    t2_d = nc.declare_dram_parameter("t2", [128, 2048], bf16, isOutput=False)
    t2r_d = nc.declare_dram_parameter("t2r", [128, 2048], bf16, isOutput=False)
    ones_d = nc.declare_dram_parameter("ones_r", [1, 128], bf16, isOutput=False)
    mhat_d = nc.declare_dram_parameter("mhat", [128, 8], f32, isOutput=False)
    id8_d = nc.declare_dram_parameter("id8_h", [128, 128], fp8, isOutput=False)
    idf_d = nc.declare_dram_parameter("idf_h", [128, 128], f32, isOutput=False)
    out_d = nc.declare_dram_parameter("out", [S, D], f32, isOutput=True)

    Exp = mybir.ActivationFunctionType.Exp
    Ident = mybir.ActivationFunctionType.Identity

    with tile.TileContext(nc) as tc:
        with tc.tile_pool(name="const", bufs=1) as cpool, \
             tc.tile_pool(name="wqks", bufs=2) as wqkpool, \
             tc.tile_pool(name="qk", bufs=2) as qkpool, \
             tc.tile_pool(name="stg", bufs=int(os.environ.get("K_STG", "10"))) as stgpool, \
             tc.tile_pool(name="skew", bufs=int(os.environ.get("K_SKEW", "2"))) as skpool, \
             tc.tile_pool(name="probs", bufs=int(os.environ.get("K_PR", "6"))) as prpool, \
             tc.tile_pool(name="ctxsb", bufs=int(os.environ.get("K_CS", "2"))) as cspool, \
             tc.tile_pool(name="osmall", bufs=4) as osmall, \
             tc.tile_pool(name="dram", bufs=16, space="DRAM") as dpool, \
             tc.tile_pool(name="qkps", bufs=int(os.environ.get("K_QKPS", "2")), space="PSUM") as qkps, \
             tc.tile_pool(name="winps", bufs=int(os.environ.get("K_WIN", "2")), space="PSUM") as winps, \
             tc.tile_pool(name="scps", bufs=int(os.environ.get("K_SC", "2")), space="PSUM") as scps, \
             tc.tile_pool(name="ctxps", bufs=2, space="PSUM") as ctxps:

            # ---- constants (ordered so pair-0 dependencies transfer first) --
            # hs^T via DMA transpose (two s-halves so 3a's first matmuls can
            # start on half 0): hsT[p, c*1024 + s] = hs[s, 128c + p]
            hsT = cpool.tile([128, 8192], bf16, tag="hsT", name="hsT")
            bqk_sb = cpool.tile([128, 16], f32, tag="bqk", name="bqk")

            def emit_hsT(qs):
                # one s-quarter of the transpose: 256 rows of hs
                nc.sync.dma_start_transpose(
                    hsT[:].__replace__(
                        ap=[[8192, 128], [1024, 8], [1, 256]],
                        offset=256 * qs),
                    hs_d.ap()[256 * qs:256 * (qs + 1), :])

            wq0 = [None]

            def emit_consts_rest():
                nc.sync.dma_start(id8[:], id8_d.ap())
                nc.sync.dma_start(t2_sb[:], t2_d.ap())
                nc.sync.dma_start(t2r_sb[:], t2r_d.ap())
                nc.sync.dma_start(mhat_sb[:], mhat_d.ap())
                nc.sync.dma_start(idf[:], idf_d.ap())
                nc.sync.dma_start(bv_sb[:], bv_d.ap())
                nc.sync.dma_start(ones_sb[:], ones_d.ap())
                nc.vector.memset(ones16[:], 1.0)
                nc.sync.dma_start(wv_sb[:], wv_d.ap())

            id8 = cpool.tile([128, 128], fp8, tag="id8", name="id8")
            t2_sb = cpool.tile([128, 2048], bf16, tag="t2", name="t2")
            t2r_sb = cpool.tile([128, 2048], bf16, tag="t2r", name="t2r")
            mhat_sb = cpool.tile([128, 8], f32, tag="mh", name="mh")
            idf = cpool.tile([128, 128], f32, tag="idf", name="idf")
            bv_sb = cpool.tile([1, 1024], bf16, tag="bv", name="bv")
            ones_sb = cpool.tile([1, 128], bf16, tag="ones", name="ones")
            ones16 = cpool.tile([128, 16], bf16, tag="o16", name="o16")
            wv_sb = cpool.tile([128, 8192], bf16, tag="wv", name="wv")

            vh = [cpool.tile([128, 1040], bf16, tag=f"vh{t}", name=f"vh{t}")
                  for t in range(NT)]
            # single output accumulator [p, L*1024 + col] so each output
            # stream is one 3-level-AP DMA instead of 8
            outacc = cpool.tile([128, 8192], f32, tag="oa", name="oa")

            qT_t = {}
            kT_t = {}
            qd_t = {}
            kd_t = {}
            s1_t = {}
            s2_t = {}
            copy_rr = [0]
            win_rr = [0]
            stt_rr = [0]
            stt_engines = os.environ.get("K_STT", "dve").split(",")
            # window-copy engine split: how many of each 16 go to ACT
            n_act = int(os.environ.get("K_CP_ACT", "8"))
            act_slots = set()
            for i in range(n_act):
                act_slots.add((i * 16) // n_act)

            def fetch_wq(P):
                wq = wqkpool.tile([128, 2048], bf16, tag="wqk", name="wqk")
                nc.sync.dma_start(
                    wq[:].__replace__(
                        ap=[[2048, 128], [256, 8], [1, 256]], offset=0),
                    wqk_d.ap().__replace__(
                        ap=[[256, 128], [32768, 8], [1, 256]],
                        offset=P * 262144))
                return wq

            def emit_3a(P, wq=None, fine=False):
                # -- 3a: qk projection for pair P; fine=True tiles the
                # moving operand at s-quarters so pair 0 starts as soon as
                # the first transpose quarter lands --
                if wq is None:
                    wq = fetch_wq(P)
                qT = qkpool.tile([128, 1024], bf16, tag="qT", name="qT")
                kT = qkpool.tile([128, 1024], bf16, tag="kT", name="kT")
                qT_t[P] = qT
                kT_t[P] = kT
                nq = 2 if fine else 1
                for dst, bcol, ct in ((qT, 0, P), (kT, 128, 8 + P)):
                    for half in range(2):
                        ps = qkps.tile([128, 512], f32, tag="qkp", name="qkp")
                        for qs in range(nq):
                            w = 512 // nq
                            for c in range(8):
                                nc.tensor.matmul(
                                    ps[:, w * qs:w * (qs + 1)],
                                    wq[:, 256 * c + bcol:256 * c + bcol + 128],
                                    hsT[:, 1024 * c + 512 * half + w * qs:
                                        1024 * c + 512 * half + w * (qs + 1)],
                                    start=(qs == 0 and c == 0),
                                    stop=(qs == nq - 1 and c == 7),
                                    skip_group_check=True)
                        nc.scalar.activation(
                            dst[:, 512 * half:512 * half + 512], ps[:], Ident,
                            bias=bqk_sb[:, ct:ct + 1], scale=1.0)
                qd = [dpool.tile([128, 9216], fp8, tag=f"qd{hh}",
                                 name=f"qd{hh}") for hh in range(2)]
                kd = [dpool.tile([128, 9216], fp8, tag=f"kd{hh}",
                                 name=f"kd{hh}") for hh in range(2)]
                qd_t[P] = qd
                kd_t[P] = kd

            def emit_3b_chunk(P, t, kind):
                # -- windows for pair P, block t, one side (hh-interleaved
                # matmuls land on row groups 0-1/2-3 for HW concurrency) --
                # k-side (kind=1) windows are stored pre-scaled by 0.125 so
                # the score-side fold is pr = (1 + s2)*exp(...), replacing
                # the former id8xs2 PE matmul-add (first-order exp identity;
                # |0.125*Wk| ~ 0.013 makes the quadratic term negligible).
                base = 896 - 128 * t
                if kind == 0:
                    src, tbl, dst_list = qT_t[P], t2r_sb, qd_t[P]
                else:
                    src, tbl, dst_list = kT_t[P], t2_sb, kd_t[P]
                scale = 1.0 if kind == 0 else 0.125
                stg = [stgpool.tile([128, WIN], fp8, tag="stg", name="stg")
                       for _ in range(2)]
                for c3 in range(3):
                    for hh in range(2):
                        rs = slice(64 * hh, 64 * (hh + 1))
                        # rotate window PSUM through both the win and the
                        # (mostly idle here) qkp tags -> 4 effective slots
                        wpool, wtag = ((winps, "win") if win_rr[0] % 2 == 0
                                       else (qkps, "qkp"))
                        win_rr[0] += 1
                        wp = wpool.tile([128, 384], f32, tag=wtag, name="win")
                        nc.tensor.matmul(
                            wp[:], src[rs, 128 * t:128 * (t + 1)],
                            tbl[rs, base + 384 * c3:base + 384 * (c3 + 1)],
                            start=True, stop=True,
                            skip_group_check=True)
                        # copy emitted right after its matmul (on ACT or DVE,
                        # alternating) so the PSUM slot recycles one matmul
                        # earlier; PE-stream hh-adjacency is unaffected.
                        sl = slice(384 * c3, 384 * (c3 + 1))
                        cnt = copy_rr[0]
                        copy_rr[0] += 1
                        if cnt % 16 in act_slots:
                            if scale == 1.0:
                                nc.scalar.copy(stg[hh][:, sl], wp[:])
                            else:
                                nc.scalar.mul(stg[hh][:, sl], wp[:], scale)
                        else:
                            if scale == 1.0:
                                nc.vector.tensor_copy(stg[hh][:, sl], wp[:])
                            else:
                                nc.vector.tensor_scalar_mul(
                                    stg[hh][:, sl], wp[:], scale)
                for hh in range(2):
                    nc.sync.dma_start(
                        dst_list[hh][:].__replace__(
                            ap=[[9216, 128], [1, WIN]], offset=WIN * t),
                        stg[hh][:])

            def emit_skew(P):
                # -- merged skew reads for pair P --
                s1p = []
                s2p = []
                for hh in range(2):
                    s1 = skpool.tile([128, 8192], fp8, tag=f"s1h{hh}",
                                     name=f"s1h{hh}")
                    nc.sync.dma_start(
                        s1[:].__replace__(
                            ap=[[8192, 128], [1024, 8], [1, 1024]], offset=0),
                        qd_t[P][hh][:].__replace__(
                            ap=[[9215, 128], [WIN, 8], [1, 1024]], offset=127))
                    s1p.append(s1)
                    s2 = skpool.tile([128, 8192], fp8, tag=f"s2h{hh}",
                                     name=f"s2h{hh}")
                    nc.sync.dma_start(
                        s2[:].__replace__(
                            ap=[[8192, 128], [1024, 8], [1, 1024]], offset=0),
                        kd_t[P][hh][:].__replace__(
                            ap=[[9215, 128], [WIN, 8], [1, 1024]], offset=127))
                    s2p.append(s2)
                s1_t[P] = s1p
                s2_t[P] = s2p

            def emit_phase2_group(tau, half):
                # one v-hat half-tile (+ones col, +bias), from resident WV
                if True:
                    if True:
                        psv = qkps.tile([128, 512], f32, tag="qkp", name="vps")
                        sl = slice(512 * half, 512 * (half + 1))
                        nc.tensor.matmul(psv[:], ones_sb[:], bv_sb[:, sl],
                                         start=True, stop=False,
                                         skip_group_check=True)
                        for c in range(8):
                            nc.tensor.matmul(
                                psv[:],
                                hsT[:, 1024 * c + 128 * tau:
                                    1024 * c + 128 * (tau + 1)],
                                wv_sb[:, 1024 * c + sl.start:
                                      1024 * c + sl.stop],
                                start=False, stop=(c == 7),
                                skip_group_check=True)
                        out_ap = vh[tau][:].__replace__(
                            ap=[[1040, 128], [65, 8], [1, 64]],
                            offset=65 * 8 * half)
                        in_ap = psv[:].__replace__(
                            ap=[[512, 128], [64, 8], [1, 64]], offset=0)
                        nc.scalar.copy(out_ap, in_ap)
                    if half == 1:
                        ones_ap = vh[tau][:].__replace__(
                            ap=[[1040, 128], [65, 16]], offset=64)
                        nc.scalar.copy(ones_ap, ones16[:])

            pc_t = {}
            ctx_pending = []

            def flush_ctx(n_keep):
                while len(ctx_pending) > n_keep:
                    pc_tile, vh_ap, pr, t = ctx_pending.pop(0)
                    nc.tensor.matmul(
                        pc_tile[:], vh_ap, pr[:],
                        start=(t == 0), stop=(t == NT - 1),
                        skip_group_check=True)

            def emit_3d_chunk(P, hh, t, lh_list=(0, 1)):
                qT = qT_t[P]
                kT = kT_t[P]
                h = 2 * P + hh
                rs = slice(64 * hh, 64 * (hh + 1))
                s1 = s1_t[P][hh]
                s2 = s2_t[P][hh]
                if t == 0 and 0 in lh_list:
                    pc_t[(P, hh)] = [
                        ctxps.tile([65, 512], f32, tag="pc", name="pc")
                        for _ in range(2)]
                pc = pc_t[(P, hh)]
                for lh in lh_list:
                    sc = scps.tile([128, 512], f32, tag="sc", name="sc")
                    for Lh in range(4):
                        L = 4 * lh + Lh
                        nc.tensor.matmul(
                            sc[:, 128 * Lh:128 * (Lh + 1)],
                            s1[:, 1024 * L + 128 * t:
                               1024 * L + 128 * (t + 1)],
                            id8[:],
                            start=(Lh == 0), stop=False,
                            skip_group_check=True)
                    nc.tensor.matmul(
                        sc[:], kT[rs, 128 * t:128 * (t + 1)],
                        qT[rs, 512 * lh:512 * (lh + 1)],
                        start=False, stop=True,
                        skip_group_check=True)
                    prp = prpool.tile([128, 512], bf16, tag="prp", name="prp")
                    nc.scalar.activation(prp[:], sc[:], Exp,
                                         bias=mhat_sb[:, t:t + 1],
                                         scale=0.125)
                    # fold the (pre-scaled) k-window via first-order exp:
                    # pr = (1 + s2)*exp(0.125*(qk + s1T) + mask), on the
                    # otherwise-idle GpSimd engine (or DVE via K_STT).
                    pr = prpool.tile([128, 512], bf16, tag="pr", name="pr")
                    stt_cnt = stt_rr[0]
                    stt_rr[0] += 1
                    eng = (nc.vector if stt_engines[stt_cnt % len(stt_engines)]
                           == "dve" else nc.gpsimd)
                    eng.scalar_tensor_tensor(
                        pr[:],
                        s2[:, 1024 * t + 512 * lh:1024 * t + 512 * (lh + 1)],
                        1.0, prp[:],
                        mybir.AluOpType.add, mybir.AluOpType.mult)
                    # ctx matmul deferred by 2 score-groups so PE never
                    # waits on the exp -> pr dependency.
                    ctx_pending.append(
                        (pc[lh], vh[t][:, 65 * h:65 * (h + 1)], pr, t))
                    flush_ctx(int(os.environ.get("K_FLUSH", "4")))

            def emit_head_tail(P, hh):
                h = 2 * P + hh
                flush_ctx(0)
                pc = pc_t.pop((P, hh))
                cs = cspool.tile([65, 1024], f32, tag="cs", name="cs")
                for lh in range(2):
                    nc.vector.tensor_copy(cs[:, 512 * lh:512 * (lh + 1)],
                                          pc[lh][:])
                # Batched output transposes: 4 L-blocks per PSUM tile from
                # the (idle here) qkp tag, one strided reciprocal per tile,
                # so PE never interlocks with the DVE drain per column.
                for g in range(2):
                    po = qkps.tile([128, 512], f32, tag="qkp", name="po")
                    for Lh in range(4):
                        L = 4 * g + Lh
                        nc.tensor.transpose(
                            po[:, 65 * Lh:65 * Lh + 65],
                            cs[:, 128 * L:128 * (L + 1)],
                            idf[0:65, 0:65])
                    rec = osmall.tile([128, 4], f32, tag="rec", name="rec")
                    nc.vector.reciprocal(
                        rec[:],
                        po[:].__replace__(ap=[[512, 128], [65, 4]],
                                          offset=64))
                    for Lh in range(4):
                        L = 4 * g + Lh
                        nc.vector.tensor_scalar_mul(
                            outacc[:, 1024 * L + 64 * h:
                                   1024 * L + 64 * (h + 1)],
                            po[:, 65 * Lh:65 * Lh + 64],
                            rec[:, Lh:Lh + 1])

            def emit_pair_cleanup(P):
                qT_t.pop(P)
                kT_t.pop(P)
                s1_t.pop(P)
                s2_t.pop(P)
                qd_t.pop(P)
                kd_t.pop(P)

            # Software-pipelined emission: pair P+1's projection and window
            # half-chunks interleave 1:1 with pair P's 16 score chunks.
            # DMACopy <-> DMATranspose transitions serialize the DMA
            # pipeline (xbar-mode workaround), so issue all plain copies
            # first, then all four transpose quarters: one transition total.
            wq0[0] = fetch_wq(0)
            for qs in (0, 1, 2, 3):
                emit_hsT(qs)
            nc.sync.dma_start(bqk_sb[:], bqk_d.ap())
            emit_3a(0, wq0[0], fine=True)
            emit_consts_rest()
            # pair-0 window chunks interleave 1:1 with the v-projection
            # groups so their PSUM->fp8 copy drain hides behind v matmuls
            for i in range(16):
                emit_3b_chunk(0, i // 2, i % 2)
                emit_phase2_group(i // 2, i % 2)
            emit_skew(0)
            for P in range(npair):
                if P + 1 < npair:
                    emit_3a(P + 1)
                for hh in range(2):
                    for t in range(NT):
                        i = hh * NT + t
                        # window chunks front-loaded into the first 8 score
                        # iterations (split around the two score groups so
                        # their PSUM copies drain while PE does score work);
                        # the skew round trip of pair P+1 then has the
                        # remaining 8 iterations of cover
                        if P + 1 < npair and i < NT:
                            emit_3b_chunk(P + 1, i, 0)
                            emit_3d_chunk(P, hh, t, (0,))
                            emit_3b_chunk(P + 1, i, 1)
                            emit_3d_chunk(P, hh, t, (1,))
                        else:
                            emit_3d_chunk(P, hh, t)
                    if hh == 0 and P + 1 < npair:
                        emit_skew(P + 1)
                    emit_head_tail(P, hh)
                emit_pair_cleanup(P)
                if P in (1, 3, 5):
                    # output columns for finished head quads are final:
                    # stream them while later pairs compute (one DMA)
                    c0 = 256 * ((P - 1) // 2)
                    nc.sync.dma_start(
                        out_d.ap().__replace__(
                            ap=[[1024, 128], [131072, 8], [1, 256]],
                            offset=c0),
                        outacc[:].__replace__(
                            ap=[[8192, 128], [1024, 8], [1, 256]],
                            offset=c0))

            for g in range(2):
                nc.sync.dma_start(
                    out_d.ap().__replace__(
                        ap=[[1024, 128], [131072, 4], [1, 256]],
                        offset=768 + 4 * g * 131072),
                    outacc[:].__replace__(
                        ap=[[8192, 128], [1024, 4], [1, 256]],
                        offset=768 + 4 * g * 1024))

    nc.compile()
    return nc


_NC_CACHE = {}
_LAST = {"exec_time_ns": None, "results": None}


def _get_program():
    if "nc" not in _NC_CACHE:
        _NC_CACHE["nc"] = build_program()
    return _NC_CACHE["nc"]


def get_last_exec_time_ns():
    return _LAST["exec_time_ns"]


def get_last_results():
    return _LAST["results"]


def kernel(hidden_states, attention_mask, W_qkv, b_qkv, dist_emb):
    from concourse.bass_utils import run_bass_kernel_spmd

    hidden_states = np.asarray(hidden_states, dtype=np.float32)
    attention_mask = np.asarray(attention_mask, dtype=np.float32)
    W_qkv = np.asarray(W_qkv, dtype=np.float32)
    b_qkv = np.asarray(b_qkv, dtype=np.float32)
    dist_emb = np.asarray(dist_emb, dtype=np.float32)

    B = hidden_states.shape[0]
    nc = _get_program()
    in_maps = host_prep(hidden_states, attention_mask, W_qkv, b_qkv, dist_emb)
    trace = bool(os.environ.get("BASS_TRACE"))
    tracedir = os.environ.get("BASS_TRACE_DIR") or None
    try:
        res = run_bass_kernel_spmd(
            nc, in_maps, list(range(B)), trace=trace, tmpdir=tracedir)
    except ModuleNotFoundError:
        res = run_bass_kernel_spmd(nc, in_maps, list(range(B)), trace=False)
    _LAST["exec_time_ns"] = res.exec_time_ns
    _LAST["results"] = res
    out = np.stack([res.results[i]["out"] for i in range(B)], axis=0)
    return out.astype(np.float32)



# revision 17
# speedup vs baseline: 1.3842x; 1.3842x over previous
"""Self-contained Trainium2 Bass kernel for BertSelfAttention (relative_key_query).

kernel(**inputs) takes FULL unsharded inputs (as in setup_inputs()) and returns
the FULL (8, 1024, 1024) float32 output. Internally: data-parallel over the
batch dimension, one batch per NeuronCore across 8 cores, via
concourse run_bass_kernel_spmd.

v3: bf16 weights/activations (fp8 only for the relative-position window
tiles, whose small magnitude makes fp8 quantization negligible), DMA-transpose
for hs^T, resident WV, packed per-pair WQK streams, merged skew-read DMAs,
1-bank score tiles, and a software-pipelined emission order interleaving pair
P+1's projection/window chunks with pair P's score chunks so the window DRAM
round trip and the PSUM->fp8 copy drain hide behind score/ctx matmuls.
"""

import os
import numpy as np

import concourse.bacc as bacc
import concourse.mybir as mybir
import concourse.tile as tile

f32 = mybir.dt.float32
bf16 = mybir.dt.bfloat16
fp8 = mybir.dt.float8e4

S = 1024
D = 1024
H = 16
DH = 64
NT = 8
WIN = 1152
NPAIR = 8


def host_prep(hidden_states, attention_mask, W_qkv, b_qkv, dist_emb):
    import ml_dtypes

    B = hidden_states.shape[0]
    W = np.asarray(W_qkv, dtype=np.float32)
    b = np.asarray(b_qkv, dtype=np.float32)
    T = np.asarray(dist_emb, dtype=np.float32)

    # Per-pair packed QK weights: [pair][dchunk][128 d][256 cols(q128|k128)]
    qcols = np.zeros((8, 128), dtype=np.int64)
    kcols = np.zeros((8, 128), dtype=np.int64)
    for t in range(8):
        for j in range(128):
            h = 2 * t + (j >= 64)
            d = j % 64
            qcols[t, j] = h * 192 + d
            kcols[t, j] = h * 192 + 64 + d
    wqkp = np.zeros((8, 8, 128, 256), dtype=np.float32)
    for P in range(8):
        cols = np.concatenate([qcols[P], kcols[P]])
        wqkp[P] = W[:, cols].reshape(8, 128, 256)
    wqkp = np.ascontiguousarray(wqkp.reshape(8, 8 * 128 * 256)).astype(
        ml_dtypes.bfloat16)
    qk_idx = np.concatenate([qcols.reshape(-1), kcols.reshape(-1)])
    bQK = np.ascontiguousarray(b[qk_idx].reshape(16, 128).T)

    # Resident V weights: [128 d][dchunk*1024 + vcol(head-major)]
    vidx = np.array([h * 192 + 128 + d for h in range(H) for d in range(DH)])
    wvp = np.ascontiguousarray(
        W[:, vidx].reshape(8, 128, 1024).transpose(1, 0, 2).reshape(128, 8192)
    ).astype(ml_dtypes.bfloat16)
    bV = np.ascontiguousarray(b[vidx].reshape(1, 1024)).astype(
        ml_dtypes.bfloat16)

    T2 = np.zeros((128, 2048), dtype=np.float32)
    T2[0:64, 0:2047] = T.T
    T2[64:128, 0:2047] = T.T
    T2R = np.zeros((128, 2048), dtype=np.float32)
    T2R[0:64, 0:2047] = T.T[:, ::-1]
    T2R[64:128, 0:2047] = T.T[:, ::-1]
    T2 = T2.astype(ml_dtypes.bfloat16)
    T2R = T2R.astype(ml_dtypes.bfloat16)

    ones_r = np.ones((1, 128), dtype=np.float32).astype(ml_dtypes.bfloat16)
    id8_h = np.eye(128, dtype=np.float32).astype(ml_dtypes.float8_e4m3fn)
    idf_h = np.eye(128, dtype=np.float32)

    mask = np.asarray(attention_mask, dtype=np.float32).reshape(B, S)
    in_maps = []
    for bi in range(B):
        mhat = np.ascontiguousarray(mask[bi].reshape(8, 128).T)
        hsb = np.ascontiguousarray(
            np.asarray(hidden_states[bi], dtype=np.float32)
        ).astype(ml_dtypes.bfloat16)
        in_maps.append({
            "hs": hsb,
            "wqkp": wqkp, "bqk": bQK, "wv": wvp, "bv": bV,
            "t2": T2, "t2r": T2R, "ones_r": ones_r, "mhat": mhat,
            "id8_h": id8_h, "idf_h": idf_h,
        })
    return in_maps


def build_program(npair=NPAIR):
    nc = bacc.Bacc()
    hs_d = nc.declare_dram_parameter("hs", [S, D], bf16, isOutput=False)
    wqk_d = nc.declare_dram_parameter("wqkp", [8, 8 * 128 * 256], bf16,
                                      isOutput=False)
    bqk_d = nc.declare_dram_parameter("bqk", [128, 16], f32, isOutput=False)
    wv_d = nc.declare_dram_parameter("wv", [128, 8192], bf16, isOutput=False)
    bv_d = nc.declare_dram_parameter("bv", [1, 1024], bf16, isOutput=False)
    t2_d = nc.declare_dram_parameter("t2", [128, 2048], bf16, isOutput=False)
    t2r_d = nc.declare_dram_parameter("t2r", [128, 2048], bf16, isOutput=False)
    ones_d = nc.declare_dram_parameter("ones_r", [1, 128], bf16, isOutput=False)
    mhat_d = nc.declare_dram_parameter("mhat", [128, 8], f32, isOutput=False)
    id8_d = nc.declare_dram_parameter("id8_h", [128, 128], fp8, isOutput=False)
    idf_d = nc.declare_dram_parameter("idf_h", [128, 128], f32, isOutput=False)
    out_d = nc.declare_dram_parameter("out", [S, D], f32, isOutput=True)

    Exp = mybir.ActivationFunctionType.Exp
    Ident = mybir.ActivationFunctionType.Identity

    with tile.TileContext(nc) as tc:
        with tc.tile_pool(name="const", bufs=1) as cpool, \
             tc.tile_pool(name="wqks", bufs=2) as wqkpool, \
             tc.tile_pool(name="qk", bufs=2) as qkpool, \
             tc.tile_pool(name="stg", bufs=int(os.environ.get("K_STG", "10"))) as stgpool, \
             tc.tile_pool(name="skew", bufs=int(os.environ.get("K_SKEW", "2"))) as skpool, \
             tc.tile_pool(name="probs", bufs=int(os.environ.get("K_PR", "6"))) as prpool, \
             tc.tile_pool(name="ctxsb", bufs=int(os.environ.get("K_CS", "2"))) as cspool, \
             tc.tile_pool(name="osmall", bufs=4) as osmall, \
             tc.tile_pool(name="dram", bufs=16, space="DRAM") as dpool, \
             tc.tile_pool(name="qkps", bufs=int(os.environ.get("K_QKPS", "2")), space="PSUM") as qkps, \
             tc.tile_pool(name="winps", bufs=int(os.environ.get("K_WIN", "2")), space="PSUM") as winps, \
             tc.tile_pool(name="scps", bufs=int(os.environ.get("K_SC", "2")), space="PSUM") as scps, \
             tc.tile_pool(name="ctxps", bufs=2, space="PSUM") as ctxps:

            # ---- constants (ordered so pair-0 dependencies transfer first) --
            # hs^T via DMA transpose, into TWO s-half tiles so pair-0's proj
            # (which reads s-half 0 first) starts after only 2 of the 4
            # transpose quarters land: hsH[h][p, c*512 + s'] = hs[512h+s',
            # 128c + p]
            hsH = [cpool.tile([128, 4096], bf16, tag=f"hsH{h}", name=f"hsH{h}")
                   for h in range(2)]
            bqk_sb = cpool.tile([128, 16], f32, tag="bqk", name="bqk")

            def hsT_ap(c, s0, w):
                # view of hs^T columns [s0, s0+w) of c-chunk c (w <= 512 and
                # the range must not cross the s=512 half boundary)
                h = s0 // 512
                return hsH[h][:, 512 * c + (s0 - 512 * h):
                              512 * c + (s0 - 512 * h) + w]

            def emit_hsT(qs):
                # one s-quarter of the transpose: 256 rows of hs
                nc.sync.dma_start_transpose(
                    hsH[qs // 2][:].__replace__(
                        ap=[[4096, 128], [512, 8], [1, 256]],
                        offset=256 * (qs % 2)),
                    hs_d.ap()[256 * qs:256 * (qs + 1), :])

            wq0 = [None]

            def emit_consts_rest():
                nc.sync.dma_start(id8[:], id8_d.ap())
                nc.sync.dma_start(t2_sb[:], t2_d.ap())
                nc.sync.dma_start(t2r_sb[:], t2r_d.ap())
                nc.sync.dma_start(mhat_sb[:], mhat_d.ap())
                nc.sync.dma_start(idf[:], idf_d.ap())
                nc.sync.dma_start(bv_sb[:], bv_d.ap())
                nc.sync.dma_start(ones_sb[:], ones_d.ap())
                nc.vector.memset(ones16[:], 1.0)
                nc.sync.dma_start(wv_sb[:], wv_d.ap())

            id8 = cpool.tile([128, 128], fp8, tag="id8", name="id8")
            t2_sb = cpool.tile([128, 2048], bf16, tag="t2", name="t2")
            t2r_sb = cpool.tile([128, 2048], bf16, tag="t2r", name="t2r")
            mhat_sb = cpool.tile([128, 8], f32, tag="mh", name="mh")
            idf = cpool.tile([128, 128], f32, tag="idf", name="idf")
            bv_sb = cpool.tile([1, 1024], bf16, tag="bv", name="bv")
            ones_sb = cpool.tile([1, 128], bf16, tag="ones", name="ones")
            ones16 = cpool.tile([128, 16], bf16, tag="o16", name="o16")
            wv_sb = cpool.tile([128, 8192], bf16, tag="wv", name="wv")

            vh = [cpool.tile([128, 1040], bf16, tag=f"vh{t}", name=f"vh{t}")
                  for t in range(NT)]
            # single output accumulator [p, L*1024 + col] so each output
            # stream is one 3-level-AP DMA instead of 8
            outacc = cpool.tile([128, 8192], f32, tag="oa", name="oa")

            qT_t = {}
            kT_t = {}
            qd_t = {}
            kd_t = {}
            s1_t = {}
            s2_t = {}
            copy_rr = [0]
            win_rr = [0]
            # window-copy engine split: how many of each 16 go to ACT
            n_act = int(os.environ.get("K_CP_ACT", "6"))
            if n_act == 6:
                act_slots = {0, 3, 5, 8, 11, 13}
            else:
                act_slots = {(i * 16) // n_act for i in range(n_act)}

            def fetch_wq(P):
                wq = wqkpool.tile([128, 2048], bf16, tag="wqk", name="wqk")
                nc.sync.dma_start(
                    wq[:].__replace__(
                        ap=[[2048, 128], [256, 8], [1, 256]], offset=0),
                    wqk_d.ap().__replace__(
                        ap=[[256, 128], [32768, 8], [1, 256]],
                        offset=P * 262144))
                return wq

            def emit_3a(P, wq=None, fine=False):
                # -- 3a: qk projection for pair P; fine=True tiles the
                # moving operand at s-quarters so pair 0 starts as soon as
                # the first transpose quarter lands --
                if wq is None:
                    wq = fetch_wq(P)
                qT = qkpool.tile([128, 1024], bf16, tag="qT", name="qT")
                kT = qkpool.tile([128, 1024], bf16, tag="kT", name="kT")
                qT_t[P] = qT
                kT_t[P] = kT
                nq = 2 if fine else 1
                for dst, bcol, ct in ((qT, 0, P), (kT, 128, 8 + P)):
                    for half in range(2):
                        ps = qkps.tile([128, 512], f32, tag="qkp", name="qkp")
                        for qs in range(nq):
                            w = 512 // nq
                            for c in range(8):
                                nc.tensor.matmul(
                                    ps[:, w * qs:w * (qs + 1)],
                                    wq[:, 256 * c + bcol:256 * c + bcol + 128],
                                    hsT_ap(c, 512 * half + w * qs, w),
                                    start=(qs == 0 and c == 0),
                                    stop=(qs == nq - 1 and c == 7),
                                    skip_group_check=True)
                        nc.scalar.activation(
                            dst[:, 512 * half:512 * half + 512], ps[:], Ident,
                            bias=bqk_sb[:, ct:ct + 1], scale=1.0)
                qd = [dpool.tile([128, 9216], fp8, tag=f"qd{hh}",
                                 name=f"qd{hh}") for hh in range(2)]
                kd = [dpool.tile([128, 9216], fp8, tag=f"kd{hh}",
                                 name=f"kd{hh}") for hh in range(2)]
                qd_t[P] = qd
                kd_t[P] = kd

            def emit_3b_chunk(P, t, kind):
                # -- windows for pair P, block t, one side (hh-interleaved
                # matmuls land on row groups 0-1/2-3 for HW concurrency) --
                # k-side (kind=1) windows are stored pre-scaled by 0.125 so
                # the score-side fold is pr = (1 + s2)*exp(...), replacing
                # the former id8xs2 PE matmul-add (first-order exp identity;
                # |0.125*Wk| ~ 0.013 makes the quadratic term negligible).
                base = 896 - 128 * t
                if kind == 0:
                    src, tbl, dst_list = qT_t[P], t2r_sb, qd_t[P]
                else:
                    src, tbl, dst_list = kT_t[P], t2_sb, kd_t[P]
                scale = 1.0
                stg = [stgpool.tile([128, WIN], fp8, tag="stg", name="stg")
                       for _ in range(2)]
                for c3 in range(3):
                    for hh in range(2):
                        rs = slice(64 * hh, 64 * (hh + 1))
                        # rotate window PSUM through both the win and the
                        # (mostly idle here) qkp tags -> 4 effective slots
                        wpool, wtag = ((winps, "win") if win_rr[0] % 2 == 0
                                       else (qkps, "qkp"))
                        win_rr[0] += 1
                        wp = wpool.tile([128, 384], f32, tag=wtag, name="win")
                        nc.tensor.matmul(
                            wp[:], src[rs, 128 * t:128 * (t + 1)],
                            tbl[rs, base + 384 * c3:base + 384 * (c3 + 1)],
                            start=True, stop=True,
                            skip_group_check=True)
                        # copy emitted right after its matmul (on ACT or DVE,
                        # alternating) so the PSUM slot recycles one matmul
                        # earlier; PE-stream hh-adjacency is unaffected.
                        sl = slice(384 * c3, 384 * (c3 + 1))
                        cnt = copy_rr[0]
                        copy_rr[0] += 1
                        if cnt % 16 in act_slots:
                            if scale == 1.0:
                                nc.scalar.copy(stg[hh][:, sl], wp[:])
                            else:
                                nc.scalar.mul(stg[hh][:, sl], wp[:], scale)
                        else:
                            if scale == 1.0:
                                nc.vector.tensor_copy(stg[hh][:, sl], wp[:])
                            else:
                                nc.vector.tensor_scalar_mul(
                                    stg[hh][:, sl], wp[:], scale)
                for hh in range(2):
                    nc.sync.dma_start(
                        dst_list[hh][:].__replace__(
                            ap=[[9216, 128], [1, WIN]], offset=WIN * t),
                        stg[hh][:])

            def emit_skew(P):
                # -- merged skew reads for pair P --
                s1p = []
                s2p = []
                for hh in range(2):
                    s1 = skpool.tile([128, 8192], fp8, tag=f"s1h{hh}",
                                     name=f"s1h{hh}")
                    nc.sync.dma_start(
                        s1[:].__replace__(
                            ap=[[8192, 128], [1024, 8], [1, 1024]], offset=0),
                        qd_t[P][hh][:].__replace__(
                            ap=[[9215, 128], [WIN, 8], [1, 1024]], offset=127))
                    s1p.append(s1)
                    s2 = skpool.tile([128, 8192], fp8, tag=f"s2h{hh}",
                                     name=f"s2h{hh}")
                    nc.sync.dma_start(
                        s2[:].__replace__(
                            ap=[[8192, 128], [1024, 8], [1, 1024]], offset=0),
                        kd_t[P][hh][:].__replace__(
                            ap=[[9215, 128], [WIN, 8], [1, 1024]], offset=127))
                    s2p.append(s2)
                s1_t[P] = s1p
                s2_t[P] = s2p

            def emit_phase2_group(tau, half):
                # one v-hat half-tile (+ones col, +bias), from resident WV
                if True:
                    if True:
                        psv = qkps.tile([128, 512], f32, tag="qkp", name="vps")
                        sl = slice(512 * half, 512 * (half + 1))
                        nc.tensor.matmul(psv[:], ones_sb[:], bv_sb[:, sl],
                                         start=True, stop=False,
                                         skip_group_check=True)
                        for c in range(8):
                            nc.tensor.matmul(
                                psv[:],
                                hsT_ap(c, 128 * tau, 128),
                                wv_sb[:, 1024 * c + sl.start:
                                      1024 * c + sl.stop],
                                start=False, stop=(c == 7),
                                skip_group_check=True)
                        out_ap = vh[tau][:].__replace__(
                            ap=[[1040, 128], [65, 8], [1, 64]],
                            offset=65 * 8 * half)
                        in_ap = psv[:].__replace__(
                            ap=[[512, 128], [64, 8], [1, 64]], offset=0)
                        nc.scalar.copy(out_ap, in_ap)
                    if half == 1:
                        ones_ap = vh[tau][:].__replace__(
                            ap=[[1040, 128], [65, 16]], offset=64)
                        nc.scalar.copy(ones_ap, ones16[:])

            pc_t = {}
            ctx_pending = []

            def flush_ctx(n_keep):
                while len(ctx_pending) > n_keep:
                    pc_tile, vh_ap, pr, t = ctx_pending.pop(0)
                    nc.tensor.matmul(
                        pc_tile[:], vh_ap, pr[:],
                        start=(t == 0), stop=(t == NT - 1),
                        skip_group_check=True)

            def emit_3d_chunk(P, hh, t, lh_list=(0, 1)):
                qT = qT_t[P]
                kT = kT_t[P]
                h = 2 * P + hh
                rs = slice(64 * hh, 64 * (hh + 1))
                s1 = s1_t[P][hh]
                s2 = s2_t[P][hh]
                if t == 0 and 0 in lh_list:
                    pc_t[(P, hh)] = [
                        ctxps.tile([65, 512], f32, tag="pc", name="pc")
                        for _ in range(2)]
                pc = pc_t[(P, hh)]
                for lh in lh_list:
                    sc = scps.tile([128, 512], f32, tag="sc", name="sc")
                    for Lh in range(4):
                        L = 4 * lh + Lh
                        nc.tensor.matmul(
                            sc[:, 128 * Lh:128 * (Lh + 1)],
                            s1[:, 1024 * L + 128 * t:
                               1024 * L + 128 * (t + 1)],
                            id8[:],
                            start=(Lh == 0), stop=False,
                            skip_group_check=True)
                    nc.tensor.matmul(
                        sc[:], kT[rs, 128 * t:128 * (t + 1)],
                        qT[rs, 512 * lh:512 * (lh + 1)],
                        start=False, stop=False,
                        skip_group_check=True)
                    nc.tensor.matmul(
                        sc[:], id8[:],
                        s2[:, 1024 * t + 512 * lh:
                           1024 * t + 512 * (lh + 1)],
                        start=False, stop=True,
                        skip_group_check=True)
                    pr = prpool.tile([128, 512], bf16, tag="pr", name="pr")
                    nc.scalar.activation(pr[:], sc[:], Exp,
                                         bias=mhat_sb[:, t:t + 1],
                                         scale=0.125)
                    # ctx matmul deferred by 2 score-groups so PE never
                    # waits on the exp -> pr dependency.
                    ctx_pending.append(
                        (pc[lh], vh[t][:, 65 * h:65 * (h + 1)], pr, t))
                    flush_ctx(int(os.environ.get("K_FLUSH", "4")))

            def emit_head_tail(P, hh):
                h = 2 * P + hh
                flush_ctx(0)
                pc = pc_t.pop((P, hh))
                cs = cspool.tile([65, 1024], f32, tag="cs", name="cs")
                for lh in range(2):
                    nc.vector.tensor_copy(cs[:, 512 * lh:512 * (lh + 1)],
                                          pc[lh][:])
                # Batched output transposes: 4 L-blocks per PSUM tile from
                # the (idle here) qkp tag, one strided reciprocal per tile,
                # so PE never interlocks with the DVE drain per column.
                for g in range(2):
                    po = qkps.tile([128, 512], f32, tag="qkp", name="po")
                    for Lh in range(4):
                        L = 4 * g + Lh
                        nc.tensor.transpose(
                            po[:, 65 * Lh:65 * Lh + 65],
                            cs[:, 128 * L:128 * (L + 1)],
                            idf[0:65, 0:65])
                    rec = osmall.tile([128, 4], f32, tag="rec", name="rec")
                    nc.vector.reciprocal(
                        rec[:],
                        po[:].__replace__(ap=[[512, 128], [65, 4]],
                                          offset=64))
                    for Lh in range(4):
                        L = 4 * g + Lh
                        nc.vector.tensor_scalar_mul(
                            outacc[:, 1024 * L + 64 * h:
                                   1024 * L + 64 * (h + 1)],
                            po[:, 65 * Lh:65 * Lh + 64],
                            rec[:, Lh:Lh + 1])

            def emit_pair_cleanup(P):
                qT_t.pop(P)
                kT_t.pop(P)
                s1_t.pop(P)
                s2_t.pop(P)
                qd_t.pop(P)
                kd_t.pop(P)

            # Software-pipelined emission: pair P+1's projection and window
            # half-chunks interleave 1:1 with pair P's 16 score chunks.
            # DMACopy <-> DMATranspose transitions serialize the DMA
            # pipeline (xbar-mode workaround), so issue all plain copies
            # first, then all four transpose quarters: one transition total.
            wq0[0] = fetch_wq(0)
            for qs in (0, 1, 2, 3):
                emit_hsT(qs)
            nc.sync.dma_start(bqk_sb[:], bqk_d.ap())
            emit_3a(0, wq0[0], fine=True)
            emit_consts_rest()
            # pair-0 window chunks interleave 1:1 with the v-projection
            # groups so their PSUM->fp8 copy drain hides behind v matmuls
            for i in range(16):
                emit_3b_chunk(0, i // 2, i % 2)
                emit_phase2_group(i // 2, i % 2)
            emit_skew(0)
            for P in range(npair):
                if P + 1 < npair:
                    emit_3a(P + 1)
                for hh in range(2):
                    for t in range(NT):
                        i = hh * NT + t
                        # window chunks front-loaded into the first 8 score
                        # iterations (split around the two score groups so
                        # their PSUM copies drain while PE does score work);
                        # the skew round trip of pair P+1 then has the
                        # remaining 8 iterations of cover
                        if P + 1 < npair and i < NT:
                            emit_3b_chunk(P + 1, i, 0)
                            emit_3d_chunk(P, hh, t, (0,))
                            emit_3b_chunk(P + 1, i, 1)
                            emit_3d_chunk(P, hh, t, (1,))
                        else:
                            emit_3d_chunk(P, hh, t)
                    if hh == 0 and P + 1 < npair:
                        emit_skew(P + 1)
                    emit_head_tail(P, hh)
                emit_pair_cleanup(P)
                if P in (1, 3, 5):
                    # output columns for finished head quads are final:
                    # stream them while later pairs compute (one DMA)
                    c0 = 256 * ((P - 1) // 2)
                    nc.sync.dma_start(
                        out_d.ap().__replace__(
                            ap=[[1024, 128], [131072, 8], [1, 256]],
                            offset=c0),
                        outacc[:].__replace__(
                            ap=[[8192, 128], [1024, 8], [1, 256]],
                            offset=c0))
                elif P == 6:
                    # heads 12-13 final: stream cols 768-895 during pair 7
                    nc.sync.dma_start(
                        out_d.ap().__replace__(
                            ap=[[1024, 128], [131072, 8], [1, 128]],
                            offset=768),
                        outacc[:].__replace__(
                            ap=[[8192, 128], [1024, 8], [1, 128]],
                            offset=768))

            for g in range(2):
                nc.sync.dma_start(
                    out_d.ap().__replace__(
                        ap=[[1024, 128], [131072, 4], [1, 128]],
                        offset=896 + 4 * g * 131072),
                    outacc[:].__replace__(
                        ap=[[8192, 128], [1024, 4], [1, 128]],
                        offset=896 + 4 * g * 1024))

    nc.compile()
    return nc


_NC_CACHE = {}
_LAST = {"exec_time_ns": None, "results": None}


def _get_program():
    if "nc" not in _NC_CACHE:
        _NC_CACHE["nc"] = build_program()
    return _NC_CACHE["nc"]


def get_last_exec_time_ns():
    return _LAST["exec_time_ns"]


def get_last_results():
    return _LAST["results"]


def kernel(hidden_states, attention_mask, W_qkv, b_qkv, dist_emb):
    from concourse.bass_utils import run_bass_kernel_spmd

    hidden_states = np.asarray(hidden_states, dtype=np.float32)
    attention_mask = np.asarray(attention_mask, dtype=np.float32)
    W_qkv = np.asarray(W_qkv, dtype=np.float32)
    b_qkv = np.asarray(b_qkv, dtype=np.float32)
    dist_emb = np.asarray(dist_emb, dtype=np.float32)

    B = hidden_states.shape[0]
    nc = _get_program()
    in_maps = host_prep(hidden_states, attention_mask, W_qkv, b_qkv, dist_emb)
    trace = bool(os.environ.get("BASS_TRACE"))
    tracedir = os.environ.get("BASS_TRACE_DIR") or None
    try:
        res = run_bass_kernel_spmd(
            nc, in_maps, list(range(B)), trace=trace, tmpdir=tracedir)
    except ModuleNotFoundError:
        res = run_bass_kernel_spmd(nc, in_maps, list(range(B)), trace=False)
    _LAST["exec_time_ns"] = res.exec_time_ns
    _LAST["results"] = res
    out = np.stack([res.results[i]["out"] for i in range(B)], axis=0)
    return out.astype(np.float32)



# revision 20
# speedup vs baseline: 1.4184x; 1.0248x over previous
"""Self-contained Trainium2 Bass kernel for BertSelfAttention (relative_key_query).

kernel(**inputs) takes FULL unsharded inputs (as in setup_inputs()) and returns
the FULL (8, 1024, 1024) float32 output. Internally: data-parallel over the
batch dimension, one batch per NeuronCore across 8 cores, via
concourse run_bass_kernel_spmd.

v3: bf16 weights/activations (fp8 only for the relative-position window
tiles, whose small magnitude makes fp8 quantization negligible), DMA-transpose
for hs^T, resident WV, packed per-pair WQK streams, merged skew-read DMAs,
1-bank score tiles, and a software-pipelined emission order interleaving pair
P+1's projection/window chunks with pair P's score chunks so the window DRAM
round trip and the PSUM->fp8 copy drain hide behind score/ctx matmuls.
"""

import os
import numpy as np

import concourse.bacc as bacc
import concourse.mybir as mybir
import concourse.tile as tile

f32 = mybir.dt.float32
bf16 = mybir.dt.bfloat16
fp8 = mybir.dt.float8e4

S = 1024
D = 1024
H = 16
DH = 64
NT = 8
WIN = 1152
NPAIR = 8


def host_prep(hidden_states, attention_mask, W_qkv, b_qkv, dist_emb):
    import ml_dtypes

    B = hidden_states.shape[0]
    W = np.asarray(W_qkv, dtype=np.float32)
    b = np.asarray(b_qkv, dtype=np.float32)
    T = np.asarray(dist_emb, dtype=np.float32)

    # Per-pair packed QK weights: [pair][dchunk][128 d][256 cols(q128|k128)]
    qcols = np.zeros((8, 128), dtype=np.int64)
    kcols = np.zeros((8, 128), dtype=np.int64)
    for t in range(8):
        for j in range(128):
            h = 2 * t + (j >= 64)
            d = j % 64
            qcols[t, j] = h * 192 + d
            kcols[t, j] = h * 192 + 64 + d
    wqkp = np.zeros((8, 8, 128, 256), dtype=np.float32)
    for P in range(8):
        cols = np.concatenate([qcols[P], kcols[P]])
        wqkp[P] = W[:, cols].reshape(8, 128, 256)
    wqkp = np.ascontiguousarray(wqkp.reshape(8, 8 * 128 * 256)).astype(
        ml_dtypes.bfloat16)
    qk_idx = np.concatenate([qcols.reshape(-1), kcols.reshape(-1)])
    bQK = np.ascontiguousarray(b[qk_idx].reshape(16, 128).T)

    # Resident V weights: [128 d][dchunk*1024 + vcol(head-major)]
    vidx = np.array([h * 192 + 128 + d for h in range(H) for d in range(DH)])
    wvp = np.ascontiguousarray(
        W[:, vidx].reshape(8, 128, 1024).transpose(1, 0, 2).reshape(128, 8192)
    ).astype(ml_dtypes.bfloat16)
    bV = np.ascontiguousarray(b[vidx].reshape(1, 1024)).astype(
        ml_dtypes.bfloat16)

    T2 = np.zeros((128, 2048), dtype=np.float32)
    T2[0:64, 0:2047] = T.T
    T2[64:128, 0:2047] = T.T
    T2R = np.zeros((128, 2048), dtype=np.float32)
    T2R[0:64, 0:2047] = T.T[:, ::-1]
    T2R[64:128, 0:2047] = T.T[:, ::-1]
    T2 = T2.astype(ml_dtypes.bfloat16)
    T2R = T2R.astype(ml_dtypes.bfloat16)

    ones_r = np.ones((1, 128), dtype=np.float32).astype(ml_dtypes.bfloat16)
    id8_h = np.eye(128, dtype=np.float32).astype(ml_dtypes.float8_e4m3fn)
    idf_h = np.eye(128, dtype=np.float32)

    mask = np.asarray(attention_mask, dtype=np.float32).reshape(B, S)
    in_maps = []
    for bi in range(B):
        mhat = np.ascontiguousarray(mask[bi].reshape(8, 128).T)
        hsb = np.ascontiguousarray(
            np.asarray(hidden_states[bi], dtype=np.float32)
        ).astype(ml_dtypes.bfloat16)
        in_maps.append({
            "hs": hsb,
            "wqkp": wqkp, "bqk": bQK, "wv": wvp, "bv": bV,
            "t2": T2, "t2r": T2R, "ones_r": ones_r, "mhat": mhat,
            "id8_h": id8_h, "idf_h": idf_h,
        })
    return in_maps


def build_program(npair=NPAIR):
    nc = bacc.Bacc()
    hs_d = nc.declare_dram_parameter("hs", [S, D], bf16, isOutput=False)
    wqk_d = nc.declare_dram_parameter("wqkp", [8, 8 * 128 * 256], bf16,
                                      isOutput=False)
    bqk_d = nc.declare_dram_parameter("bqk", [128, 16], f32, isOutput=False)
    wv_d = nc.declare_dram_parameter("wv", [128, 8192], bf16, isOutput=False)
    bv_d = nc.declare_dram_parameter("bv", [1, 1024], bf16, isOutput=False)
    t2_d = nc.declare_dram_parameter("t2", [128, 2048], bf16, isOutput=False)
    t2r_d = nc.declare_dram_parameter("t2r", [128, 2048], bf16, isOutput=False)
    ones_d = nc.declare_dram_parameter("ones_r", [1, 128], bf16, isOutput=False)
    mhat_d = nc.declare_dram_parameter("mhat", [128, 8], f32, isOutput=False)
    id8_d = nc.declare_dram_parameter("id8_h", [128, 128], fp8, isOutput=False)
    idf_d = nc.declare_dram_parameter("idf_h", [128, 128], f32, isOutput=False)
    out_d = nc.declare_dram_parameter("out", [S, D], f32, isOutput=True)

    Exp = mybir.ActivationFunctionType.Exp
    Ident = mybir.ActivationFunctionType.Identity

    with tile.TileContext(nc) as tc:
        with tc.tile_pool(name="const", bufs=1) as cpool, \
             tc.tile_pool(name="wqks", bufs=2) as wqkpool, \
             tc.tile_pool(name="qk", bufs=2) as qkpool, \
             tc.tile_pool(name="stg", bufs=int(os.environ.get("K_STG", "10"))) as stgpool, \
             tc.tile_pool(name="skew", bufs=int(os.environ.get("K_SKEW", "2"))) as skpool, \
             tc.tile_pool(name="probs", bufs=int(os.environ.get("K_PR", "6"))) as prpool, \
             tc.tile_pool(name="ctxsb", bufs=int(os.environ.get("K_CS", "2"))) as cspool, \
             tc.tile_pool(name="osmall", bufs=4) as osmall, \
             tc.tile_pool(name="dram", bufs=16, space="DRAM") as dpool, \
             tc.tile_pool(name="qkps", bufs=int(os.environ.get("K_QKPS", "2")), space="PSUM") as qkps, \
             tc.tile_pool(name="winps", bufs=int(os.environ.get("K_WIN", "2")), space="PSUM") as winps, \
             tc.tile_pool(name="scps", bufs=int(os.environ.get("K_SC", "2")), space="PSUM") as scps, \
             tc.tile_pool(name="ctxps", bufs=2, space="PSUM") as ctxps:

            # ---- constants (ordered so pair-0 dependencies transfer first) --
            # hs^T via DMA transpose, into TWO s-half tiles so pair-0's proj
            # (which reads s-half 0 first) starts after only 2 of the 4
            # transpose quarters land: hsH[h][p, c*512 + s'] = hs[512h+s',
            # 128c + p]
            hsH = [cpool.tile([128, 4096], bf16, tag=f"hsH{h}", name=f"hsH{h}")
                   for h in range(2)]
            bqk_sb = cpool.tile([128, 16], f32, tag="bqk", name="bqk")

            def hsT_ap(c, s0, w):
                # view of hs^T columns [s0, s0+w) of c-chunk c (w <= 512 and
                # the range must not cross the s=512 half boundary)
                h = s0 // 512
                return hsH[h][:, 512 * c + (s0 - 512 * h):
                              512 * c + (s0 - 512 * h) + w]

            def emit_hsT(qs):
                # one s-quarter of the transpose: 256 rows of hs
                nc.sync.dma_start_transpose(
                    hsH[qs // 2][:].__replace__(
                        ap=[[4096, 128], [512, 8], [1, 256]],
                        offset=256 * (qs % 2)),
                    hs_d.ap()[256 * qs:256 * (qs + 1), :])

            wq0 = [None]

            def emit_consts_rest():
                nc.sync.dma_start(id8[:], id8_d.ap())
                nc.sync.dma_start(t2_sb[:], t2_d.ap())
                nc.sync.dma_start(t2r_sb[:], t2r_d.ap())
                nc.sync.dma_start(mhat_sb[:], mhat_d.ap())
                nc.sync.dma_start(idf[:], idf_d.ap())
                nc.sync.dma_start(bv_sb[:], bv_d.ap())
                nc.sync.dma_start(ones_sb[:], ones_d.ap())
                nc.vector.memset(ones16[:], 1.0)
                nc.sync.dma_start(wv_sb[:], wv_d.ap())

            id8 = cpool.tile([128, 128], fp8, tag="id8", name="id8")
            t2_sb = cpool.tile([128, 2048], bf16, tag="t2", name="t2")
            t2r_sb = cpool.tile([128, 2048], bf16, tag="t2r", name="t2r")
            mhat_sb = cpool.tile([128, 8], f32, tag="mh", name="mh")
            idf = cpool.tile([128, 128], f32, tag="idf", name="idf")
            bv_sb = cpool.tile([1, 1024], bf16, tag="bv", name="bv")
            ones_sb = cpool.tile([1, 128], bf16, tag="ones", name="ones")
            ones16 = cpool.tile([128, 16], bf16, tag="o16", name="o16")
            wv_sb = cpool.tile([128, 8192], bf16, tag="wv", name="wv")

            vh = [cpool.tile([128, 1040], bf16, tag=f"vh{t}", name=f"vh{t}")
                  for t in range(NT)]
            # single output accumulator [p, L*1024 + col] so each output
            # stream is one 3-level-AP DMA instead of 8
            outacc = cpool.tile([128, 8192], f32, tag="oa", name="oa")

            qT_t = {}
            kT_t = {}
            qd_t = {}
            kd_t = {}
            s1_t = {}
            s2_t = {}
            copy_rr = [0]
            win_rr = [0]
            # window-copy engine split: how many of each 16 go to ACT
            n_act = int(os.environ.get("K_CP_ACT", "6"))
            if n_act == 6:
                act_slots = {0, 3, 5, 8, 11, 13}
            else:
                act_slots = {(i * 16) // n_act for i in range(n_act)}

            def fetch_wq(P):
                wq = wqkpool.tile([128, 2048], bf16, tag="wqk", name="wqk")
                nc.sync.dma_start(
                    wq[:].__replace__(
                        ap=[[2048, 128], [256, 8], [1, 256]], offset=0),
                    wqk_d.ap().__replace__(
                        ap=[[256, 128], [32768, 8], [1, 256]],
                        offset=P * 262144))
                return wq

            def emit_3a(P, wq=None, fine=False):
                # -- 3a: qk projection for pair P; fine=True tiles the
                # moving operand at s-quarters so pair 0 starts as soon as
                # the first transpose quarter lands --
                if wq is None:
                    wq = fetch_wq(P)
                qT = qkpool.tile([128, 1024], bf16, tag="qT", name="qT")
                kT = qkpool.tile([128, 1024], bf16, tag="kT", name="kT")
                qT_t[P] = qT
                kT_t[P] = kT
                nq = 2 if fine else 1
                for dst, bcol, ct in ((qT, 0, P), (kT, 128, 8 + P)):
                    for half in range(2):
                        ps = qkps.tile([128, 512], f32, tag="qkp", name="qkp")
                        for qs in range(nq):
                            w = 512 // nq
                            for c in range(8):
                                nc.tensor.matmul(
                                    ps[:, w * qs:w * (qs + 1)],
                                    wq[:, 256 * c + bcol:256 * c + bcol + 128],
                                    hsT_ap(c, 512 * half + w * qs, w),
                                    start=(qs == 0 and c == 0),
                                    stop=(qs == nq - 1 and c == 7),
                                    skip_group_check=True)
                        nc.scalar.activation(
                            dst[:, 512 * half:512 * half + 512], ps[:], Ident,
                            bias=bqk_sb[:, ct:ct + 1], scale=1.0)
                qd = [dpool.tile([128, 9216], fp8, tag=f"qd{hh}",
                                 name=f"qd{hh}") for hh in range(2)]
                kd = [dpool.tile([128, 9216], fp8, tag=f"kd{hh}",
                                 name=f"kd{hh}") for hh in range(2)]
                qd_t[P] = qd
                kd_t[P] = kd

            def emit_3b_chunk(P, t, kind):
                # -- windows for pair P, block t, one side (hh-interleaved
                # matmuls land on row groups 0-1/2-3 for HW concurrency) --
                # k-side (kind=1) windows are stored pre-scaled by 0.125 so
                # the score-side fold is pr = (1 + s2)*exp(...), replacing
                # the former id8xs2 PE matmul-add (first-order exp identity;
                # |0.125*Wk| ~ 0.013 makes the quadratic term negligible).
                base = 896 - 128 * t
                if kind == 0:
                    src, tbl, dst_list = qT_t[P], t2r_sb, qd_t[P]
                else:
                    src, tbl, dst_list = kT_t[P], t2_sb, kd_t[P]
                scale = 1.0
                stg = [stgpool.tile([128, WIN], fp8, tag="stg", name="stg")
                       for _ in range(2)]
                for c3 in range(3):
                    for hh in range(2):
                        rs = slice(64 * hh, 64 * (hh + 1))
                        # rotate window PSUM through both the win and the
                        # (mostly idle here) qkp tags -> 4 effective slots
                        wpool, wtag = ((winps, "win") if win_rr[0] % 2 == 0
                                       else (qkps, "qkp"))
                        win_rr[0] += 1
                        wp = wpool.tile([128, 384], f32, tag=wtag, name="win")
                        nc.tensor.matmul(
                            wp[:], src[rs, 128 * t:128 * (t + 1)],
                            tbl[rs, base + 384 * c3:base + 384 * (c3 + 1)],
                            start=True, stop=True,
                            skip_group_check=True)
                        # copy emitted right after its matmul (on ACT or DVE,
                        # alternating) so the PSUM slot recycles one matmul
                        # earlier; PE-stream hh-adjacency is unaffected.
                        sl = slice(384 * c3, 384 * (c3 + 1))
                        cnt = copy_rr[0]
                        copy_rr[0] += 1
                        if cnt % 16 in act_slots:
                            if scale == 1.0:
                                nc.scalar.copy(stg[hh][:, sl], wp[:])
                            else:
                                nc.scalar.mul(stg[hh][:, sl], wp[:], scale)
                        else:
                            if scale == 1.0:
                                nc.vector.tensor_copy(stg[hh][:, sl], wp[:])
                            else:
                                nc.vector.tensor_scalar_mul(
                                    stg[hh][:, sl], wp[:], scale)
                for hh in range(2):
                    nc.sync.dma_start(
                        dst_list[hh][:].__replace__(
                            ap=[[9216, 128], [1, WIN]], offset=WIN * t),
                        stg[hh][:])

            def emit_skew(P):
                # -- merged skew reads for pair P --
                s1p = []
                s2p = []
                for hh in range(2):
                    s1 = skpool.tile([128, 8192], fp8, tag=f"s1h{hh}",
                                     name=f"s1h{hh}")
                    nc.sync.dma_start(
                        s1[:].__replace__(
                            ap=[[8192, 128], [1024, 8], [1, 1024]], offset=0),
                        qd_t[P][hh][:].__replace__(
                            ap=[[9215, 128], [WIN, 8], [1, 1024]], offset=127))
                    s1p.append(s1)
                    s2 = skpool.tile([128, 8192], fp8, tag=f"s2h{hh}",
                                     name=f"s2h{hh}")
                    nc.sync.dma_start(
                        s2[:].__replace__(
                            ap=[[8192, 128], [1024, 8], [1, 1024]], offset=0),
                        kd_t[P][hh][:].__replace__(
                            ap=[[9215, 128], [WIN, 8], [1, 1024]], offset=127))
                    s2p.append(s2)
                s1_t[P] = s1p
                s2_t[P] = s2p

            def emit_phase2_group(tau, half):
                # one v-hat half-tile (+ones col, +bias), from resident WV
                if True:
                    if True:
                        psv = qkps.tile([128, 512], f32, tag="qkp", name="vps")
                        sl = slice(512 * half, 512 * (half + 1))
                        nc.tensor.matmul(psv[:], ones_sb[:], bv_sb[:, sl],
                                         start=True, stop=False,
                                         skip_group_check=True)
                        for c in range(8):
                            nc.tensor.matmul(
                                psv[:],
                                hsT_ap(c, 128 * tau, 128),
                                wv_sb[:, 1024 * c + sl.start:
                                      1024 * c + sl.stop],
                                start=False, stop=(c == 7),
                                skip_group_check=True)
                        out_ap = vh[tau][:].__replace__(
                            ap=[[1040, 128], [65, 8], [1, 64]],
                            offset=65 * 8 * half)
                        in_ap = psv[:].__replace__(
                            ap=[[512, 128], [64, 8], [1, 64]], offset=0)
                        nc.scalar.copy(out_ap, in_ap)
                    if half == 1:
                        ones_ap = vh[tau][:].__replace__(
                            ap=[[1040, 128], [65, 16]], offset=64)
                        nc.scalar.copy(ones_ap, ones16[:])

            pc_t = {}
            ctx_pending = []

            def flush_ctx(n_keep):
                while len(ctx_pending) > n_keep:
                    pc_tile, vh_ap, pr, t = ctx_pending.pop(0)
                    nc.tensor.matmul(
                        pc_tile[:], vh_ap, pr[:],
                        start=(t == 0), stop=(t == NT - 1),
                        skip_group_check=True)

            def emit_3d_chunk(P, hh, t, lh_list=(0, 1)):
                qT = qT_t[P]
                kT = kT_t[P]
                h = 2 * P + hh
                rs = slice(64 * hh, 64 * (hh + 1))
                s1 = s1_t[P][hh]
                s2 = s2_t[P][hh]
                if t == 0 and 0 in lh_list:
                    pc_t[(P, hh)] = [
                        ctxps.tile([65, 512], f32, tag="pc", name="pc")
                        for _ in range(2)]
                pc = pc_t[(P, hh)]
                # qk matmuls first: their stationary is 64-row like the
                # window matmuls that precede this chunk, avoiding a PE
                # row-tile reconfig flush (~96ns) on entry; when both lh
                # halves are emitted together, both 64-row qk matmuls are
                # grouped before the 128-row transpose/s2-add block.
                sc_t = {}
                for lh in lh_list:
                    sc = scps.tile([128, 512], f32, tag="sc", name="sc")
                    sc_t[lh] = sc
                    nc.tensor.matmul(
                        sc[:], kT[rs, 128 * t:128 * (t + 1)],
                        qT[rs, 512 * lh:512 * (lh + 1)],
                        start=True, stop=False,
                        skip_group_check=True)
                for lh in lh_list:
                    sc = sc_t[lh]
                    for Lh in range(4):
                        L = 4 * lh + Lh
                        nc.tensor.matmul(
                            sc[:, 128 * Lh:128 * (Lh + 1)],
                            s1[:, 1024 * L + 128 * t:
                               1024 * L + 128 * (t + 1)],
                            id8[:],
                            start=False, stop=False,
                            skip_group_check=True)
                    nc.tensor.matmul(
                        sc[:], id8[:],
                        s2[:, 1024 * t + 512 * lh:
                           1024 * t + 512 * (lh + 1)],
                        start=False, stop=True,
                        skip_group_check=True)
                    pr = prpool.tile([128, 512], bf16, tag="pr", name="pr")
                    nc.scalar.activation(pr[:], sc[:], Exp,
                                         bias=mhat_sb[:, t:t + 1],
                                         scale=0.125)
                    # ctx matmul deferred by 2 score-groups so PE never
                    # waits on the exp -> pr dependency.
                    ctx_pending.append(
                        (pc[lh], vh[t][:, 65 * h:65 * (h + 1)], pr, t))
                    flush_ctx(int(os.environ.get("K_FLUSH", "4")))

            def emit_head_tail(P, hh):
                h = 2 * P + hh
                flush_ctx(0)
                pc = pc_t.pop((P, hh))
                cs = cspool.tile([65, 1024], f32, tag="cs", name="cs")
                for lh in range(2):
                    nc.vector.tensor_copy(cs[:, 512 * lh:512 * (lh + 1)],
                                          pc[lh][:])
                # Batched output transposes: 4 L-blocks per PSUM tile from
                # the (idle here) qkp tag, one strided reciprocal per tile,
                # so PE never interlocks with the DVE drain per column.
                for g in range(2):
                    po = qkps.tile([128, 512], f32, tag="qkp", name="po")
                    for Lh in range(4):
                        L = 4 * g + Lh
                        nc.tensor.transpose(
                            po[:, 65 * Lh:65 * Lh + 65],
                            cs[:, 128 * L:128 * (L + 1)],
                            idf[0:65, 0:65])
                    rec = osmall.tile([128, 4], f32, tag="rec", name="rec")
                    nc.vector.reciprocal(
                        rec[:],
                        po[:].__replace__(ap=[[512, 128], [65, 4]],
                                          offset=64))
                    for Lh in range(4):
                        L = 4 * g + Lh
                        nc.vector.tensor_scalar_mul(
                            outacc[:, 1024 * L + 64 * h:
                                   1024 * L + 64 * (h + 1)],
                            po[:, 65 * Lh:65 * Lh + 64],
                            rec[:, Lh:Lh + 1])

            def emit_pair_cleanup(P):
                qT_t.pop(P)
                kT_t.pop(P)
                s1_t.pop(P)
                s2_t.pop(P)
                qd_t.pop(P)
                kd_t.pop(P)

            # Software-pipelined emission: pair P+1's projection and window
            # half-chunks interleave 1:1 with pair P's 16 score chunks.
            # DMACopy <-> DMATranspose transitions serialize the DMA
            # pipeline (xbar-mode workaround), so issue all plain copies
            # first, then all four transpose quarters: one transition total.
            wq0[0] = fetch_wq(0)
            for qs in (0, 1, 2, 3):
                emit_hsT(qs)
            nc.sync.dma_start(bqk_sb[:], bqk_d.ap())
            emit_3a(0, wq0[0], fine=True)
            emit_consts_rest()
            # pair-0 window chunks interleave 1:1 with the v-projection
            # groups so their PSUM->fp8 copy drain hides behind v matmuls
            for i in range(16):
                emit_3b_chunk(0, i // 2, i % 2)
                emit_phase2_group(i // 2, i % 2)
            emit_skew(0)
            for P in range(npair):
                if P + 1 < npair:
                    emit_3a(P + 1)
                for hh in range(2):
                    for t in range(NT):
                        i = hh * NT + t
                        # window chunks front-loaded into the first 8 score
                        # iterations (split around the two score groups so
                        # their PSUM copies drain while PE does score work);
                        # the skew round trip of pair P+1 then has the
                        # remaining 8 iterations of cover
                        if P + 1 < npair and i < NT:
                            emit_3b_chunk(P + 1, i, 0)
                            emit_3d_chunk(P, hh, t, (0,))
                            emit_3b_chunk(P + 1, i, 1)
                            emit_3d_chunk(P, hh, t, (1,))
                        else:
                            emit_3d_chunk(P, hh, t)
                    if hh == 0 and P + 1 < npair:
                        emit_skew(P + 1)
                    emit_head_tail(P, hh)
                    if P == 7 and hh == 0:
                        # head 14 final: stream its columns during head 15's
                        # score/ctx work to shorten the serial tail
                        nc.sync.dma_start(
                            out_d.ap().__replace__(
                                ap=[[1024, 128], [131072, 8], [1, 64]],
                                offset=896),
                            outacc[:].__replace__(
                                ap=[[8192, 128], [1024, 8], [1, 64]],
                                offset=896))
                emit_pair_cleanup(P)
                if P in (1, 3, 5):
                    # output columns for finished head quads are final:
                    # stream them while later pairs compute (one DMA)
                    c0 = 256 * ((P - 1) // 2)
                    nc.sync.dma_start(
                        out_d.ap().__replace__(
                            ap=[[1024, 128], [131072, 8], [1, 256]],
                            offset=c0),
                        outacc[:].__replace__(
                            ap=[[8192, 128], [1024, 8], [1, 256]],
                            offset=c0))
                elif P == 6:
                    # heads 12-13 final: stream cols 768-895 during pair 7
                    nc.sync.dma_start(
                        out_d.ap().__replace__(
                            ap=[[1024, 128], [131072, 8], [1, 128]],
                            offset=768),
                        outacc[:].__replace__(
                            ap=[[8192, 128], [1024, 8], [1, 128]],
                            offset=768))

            for g in range(2):
                nc.sync.dma_start(
                    out_d.ap().__replace__(
                        ap=[[1024, 128], [131072, 4], [1, 128]],
                        offset=896 + 4 * g * 131072),
                    outacc[:].__replace__(
                        ap=[[8192, 128], [1024, 4], [1, 128]],
                        offset=896 + 4 * g * 1024))

    nc.compile()
    return nc


_NC_CACHE = {}
_LAST = {"exec_time_ns": None, "results": None}


def _get_program():
    if "nc" not in _NC_CACHE:
        _NC_CACHE["nc"] = build_program()
    return _NC_CACHE["nc"]


def get_last_exec_time_ns():
    return _LAST["exec_time_ns"]


def get_last_results():
    return _LAST["results"]


def kernel(hidden_states, attention_mask, W_qkv, b_qkv, dist_emb):
    from concourse.bass_utils import run_bass_kernel_spmd

    hidden_states = np.asarray(hidden_states, dtype=np.float32)
    attention_mask = np.asarray(attention_mask, dtype=np.float32)
    W_qkv = np.asarray(W_qkv, dtype=np.float32)
    b_qkv = np.asarray(b_qkv, dtype=np.float32)
    dist_emb = np.asarray(dist_emb, dtype=np.float32)

    B = hidden_states.shape[0]
    nc = _get_program()
    in_maps = host_prep(hidden_states, attention_mask, W_qkv, b_qkv, dist_emb)
    trace = bool(os.environ.get("BASS_TRACE"))
    tracedir = os.environ.get("BASS_TRACE_DIR") or None
    try:
        res = run_bass_kernel_spmd(
            nc, in_maps, list(range(B)), trace=trace, tmpdir=tracedir)
    except ModuleNotFoundError:
        res = run_bass_kernel_spmd(nc, in_maps, list(range(B)), trace=False)
    _LAST["exec_time_ns"] = res.exec_time_ns
    _LAST["results"] = res
    out = np.stack([res.results[i]["out"] for i in range(B)], axis=0)
    return out.astype(np.float32)



# revision 22
# speedup vs baseline: 1.4548x; 1.0257x over previous
"""Self-contained Trainium2 Bass kernel for BertSelfAttention (relative_key_query).

kernel(**inputs) takes FULL unsharded inputs (as in setup_inputs()) and returns
the FULL (8, 1024, 1024) float32 output. Internally: data-parallel over the
batch dimension, one batch per NeuronCore across 8 cores, via
concourse run_bass_kernel_spmd.

v3: bf16 weights/activations (fp8 only for the relative-position window
tiles, whose small magnitude makes fp8 quantization negligible), DMA-transpose
for hs^T, resident WV, packed per-pair WQK streams, merged skew-read DMAs,
1-bank score tiles, and a software-pipelined emission order interleaving pair
P+1's projection/window chunks with pair P's score chunks so the window DRAM
round trip and the PSUM->fp8 copy drain hide behind score/ctx matmuls.
"""

import os
import numpy as np

import concourse.bacc as bacc
import concourse.mybir as mybir
import concourse.tile as tile

f32 = mybir.dt.float32
bf16 = mybir.dt.bfloat16
fp8 = mybir.dt.float8e4

S = 1024
D = 1024
H = 16
DH = 64
NT = 8
WIN = 1152
NPAIR = 8


def host_prep(hidden_states, attention_mask, W_qkv, b_qkv, dist_emb):
    import ml_dtypes

    B = hidden_states.shape[0]
    W = np.asarray(W_qkv, dtype=np.float32)
    b = np.asarray(b_qkv, dtype=np.float32)
    T = np.asarray(dist_emb, dtype=np.float32)

    # Per-pair packed QK weights: [pair][dchunk][128 d][256 cols(q128|k128)]
    qcols = np.zeros((8, 128), dtype=np.int64)
    kcols = np.zeros((8, 128), dtype=np.int64)
    for t in range(8):
        for j in range(128):
            h = 2 * t + (j >= 64)
            d = j % 64
            qcols[t, j] = h * 192 + d
            kcols[t, j] = h * 192 + 64 + d
    wqkp = np.zeros((8, 8, 128, 256), dtype=np.float32)
    for P in range(8):
        cols = np.concatenate([qcols[P], kcols[P]])
        wqkp[P] = W[:, cols].reshape(8, 128, 256)
    wqkp = np.ascontiguousarray(wqkp.reshape(8, 8 * 128 * 256)).astype(
        ml_dtypes.bfloat16)
    qk_idx = np.concatenate([qcols.reshape(-1), kcols.reshape(-1)])
    bQK = np.ascontiguousarray(b[qk_idx].reshape(16, 128).T)

    # Resident V weights: [128 d][dchunk*1024 + vcol(head-major)]
    vidx = np.array([h * 192 + 128 + d for h in range(H) for d in range(DH)])
    wvp = np.ascontiguousarray(
        W[:, vidx].reshape(8, 128, 1024).transpose(1, 0, 2).reshape(128, 8192)
    ).astype(ml_dtypes.bfloat16)
    bV = np.ascontiguousarray(b[vidx].reshape(1, 1024)).astype(
        ml_dtypes.bfloat16)

    T2 = np.zeros((128, 2048), dtype=np.float32)
    T2[0:64, 0:2047] = T.T
    T2[64:128, 0:2047] = T.T
    T2R = np.zeros((128, 2048), dtype=np.float32)
    T2R[0:64, 0:2047] = T.T[:, ::-1]
    T2R[64:128, 0:2047] = T.T[:, ::-1]
    T2 = T2.astype(ml_dtypes.bfloat16)
    T2R = T2R.astype(ml_dtypes.bfloat16)

    ones_r = np.ones((1, 128), dtype=np.float32).astype(ml_dtypes.bfloat16)
    id8_h = np.eye(128, dtype=np.float32).astype(ml_dtypes.float8_e4m3fn)
    idf_h = np.eye(128, dtype=np.float32)

    mask = np.asarray(attention_mask, dtype=np.float32).reshape(B, S)
    in_maps = []
    for bi in range(B):
        mhat = np.ascontiguousarray(mask[bi].reshape(8, 128).T)
        hsb = np.ascontiguousarray(
            np.asarray(hidden_states[bi], dtype=np.float32)
        ).astype(ml_dtypes.bfloat16)
        in_maps.append({
            "hs": hsb,
            "wqkp": wqkp, "bqk": bQK, "wv": wvp, "bv": bV,
            "t2": T2, "t2r": T2R, "ones_r": ones_r, "mhat": mhat,
            "id8_h": id8_h, "idf_h": idf_h,
        })
    return in_maps


def build_program(npair=NPAIR):
    nc = bacc.Bacc()
    hs_d = nc.declare_dram_parameter("hs", [S, D], bf16, isOutput=False)
    wqk_d = nc.declare_dram_parameter("wqkp", [8, 8 * 128 * 256], bf16,
                                      isOutput=False)
    bqk_d = nc.declare_dram_parameter("bqk", [128, 16], f32, isOutput=False)
    wv_d = nc.declare_dram_parameter("wv", [128, 8192], bf16, isOutput=False)
    bv_d = nc.declare_dram_parameter("bv", [1, 1024], bf16, isOutput=False)
    t2_d = nc.declare_dram_parameter("t2", [128, 2048], bf16, isOutput=False)
    t2r_d = nc.declare_dram_parameter("t2r", [128, 2048], bf16, isOutput=False)
    ones_d = nc.declare_dram_parameter("ones_r", [1, 128], bf16, isOutput=False)
    mhat_d = nc.declare_dram_parameter("mhat", [128, 8], f32, isOutput=False)
    id8_d = nc.declare_dram_parameter("id8_h", [128, 128], fp8, isOutput=False)
    idf_d = nc.declare_dram_parameter("idf_h", [128, 128], f32, isOutput=False)
    out_d = nc.declare_dram_parameter("out", [S, D], f32, isOutput=True)

    Exp = mybir.ActivationFunctionType.Exp
    Ident = mybir.ActivationFunctionType.Identity

    with tile.TileContext(nc) as tc:
        with tc.tile_pool(name="const", bufs=1) as cpool, \
             tc.tile_pool(name="wqks", bufs=2) as wqkpool, \
             tc.tile_pool(name="qk", bufs=2) as qkpool, \
             tc.tile_pool(name="stg", bufs=int(os.environ.get("K_STG", "10"))) as stgpool, \
             tc.tile_pool(name="skew", bufs=int(os.environ.get("K_SKEW", "2"))) as skpool, \
             tc.tile_pool(name="probs", bufs=int(os.environ.get("K_PR", "6"))) as prpool, \
             tc.tile_pool(name="ctxsb", bufs=int(os.environ.get("K_CS", "2"))) as cspool, \
             tc.tile_pool(name="osmall", bufs=4) as osmall, \
             tc.tile_pool(name="dram", bufs=16, space="DRAM") as dpool, \
             tc.tile_pool(name="qkps", bufs=int(os.environ.get("K_QKPS", "2")), space="PSUM") as qkps, \
             tc.tile_pool(name="winps", bufs=int(os.environ.get("K_WIN", "2")), space="PSUM") as winps, \
             tc.tile_pool(name="scps", bufs=int(os.environ.get("K_SC", "2")), space="PSUM") as scps, \
             tc.tile_pool(name="ctxps", bufs=2, space="PSUM") as ctxps:

            # ---- constants (ordered so pair-0 dependencies transfer first) --
            # hs^T via DMA transpose, into TWO s-half tiles so pair-0's proj
            # (which reads s-half 0 first) starts after only 2 of the 4
            # transpose quarters land: hsH[h][p, c*512 + s'] = hs[512h+s',
            # 128c + p]
            hsH = [cpool.tile([128, 4096], bf16, tag=f"hsH{h}", name=f"hsH{h}")
                   for h in range(2)]
            bqk_sb = cpool.tile([128, 16], f32, tag="bqk", name="bqk")

            def hsT_ap(c, s0, w):
                # view of hs^T columns [s0, s0+w) of c-chunk c (w <= 512 and
                # the range must not cross the s=512 half boundary)
                h = s0 // 512
                return hsH[h][:, 512 * c + (s0 - 512 * h):
                              512 * c + (s0 - 512 * h) + w]

            def emit_hsT(qs):
                # one s-quarter of the transpose: 256 rows of hs
                nc.sync.dma_start_transpose(
                    hsH[qs // 2][:].__replace__(
                        ap=[[4096, 128], [512, 8], [1, 256]],
                        offset=256 * (qs % 2)),
                    hs_d.ap()[256 * qs:256 * (qs + 1), :])

            wq0 = [None]

            def emit_consts_rest():
                nc.sync.dma_start(id8[:], id8_d.ap())
                nc.sync.dma_start(t2_sb[:], t2_d.ap())
                nc.sync.dma_start(t2r_sb[:], t2r_d.ap())
                nc.sync.dma_start(mhat_sb[:], mhat_d.ap())
                nc.sync.dma_start(idf[:], idf_d.ap())
                nc.sync.dma_start(bv_sb[:], bv_d.ap())
                nc.sync.dma_start(ones_sb[:], ones_d.ap())
                nc.vector.memset(ones16[:], 1.0)
                nc.sync.dma_start(wv_sb[:], wv_d.ap())

            id8 = cpool.tile([128, 128], fp8, tag="id8", name="id8")
            t2_sb = cpool.tile([128, 2048], bf16, tag="t2", name="t2")
            t2r_sb = cpool.tile([128, 2048], bf16, tag="t2r", name="t2r")
            mhat_sb = cpool.tile([128, 8], f32, tag="mh", name="mh")
            idf = cpool.tile([128, 128], f32, tag="idf", name="idf")
            bv_sb = cpool.tile([1, 1024], bf16, tag="bv", name="bv")
            ones_sb = cpool.tile([1, 128], bf16, tag="ones", name="ones")
            ones16 = cpool.tile([128, 16], bf16, tag="o16", name="o16")
            wv_sb = cpool.tile([128, 8192], bf16, tag="wv", name="wv")

            vh = [cpool.tile([128, 1040], bf16, tag=f"vh{t}", name=f"vh{t}")
                  for t in range(NT)]
            # single output accumulator [p, L*1024 + col] so each output
            # stream is one 3-level-AP DMA instead of 8
            outacc = cpool.tile([128, 8192], f32, tag="oa", name="oa")

            qT_t = {}
            kT_t = {}
            qd_t = {}
            kd_t = {}
            s1_t = {}
            s2_t = {}
            copy_rr = [0]
            win_rr = [0]
            # window-copy engine split: how many of each 16 go to ACT
            n_act = int(os.environ.get("K_CP_ACT", "6"))
            if n_act == 6:
                act_slots = {0, 3, 5, 8, 11, 13}
            else:
                act_slots = {(i * 16) // n_act for i in range(n_act)}

            def fetch_wq(P):
                wq = wqkpool.tile([128, 2048], bf16, tag="wqk", name="wqk")
                nc.sync.dma_start(
                    wq[:].__replace__(
                        ap=[[2048, 128], [256, 8], [1, 256]], offset=0),
                    wqk_d.ap().__replace__(
                        ap=[[256, 128], [32768, 8], [1, 256]],
                        offset=P * 262144))
                return wq

            def emit_3a(P, wq=None, fine=False):
                # -- 3a: qk projection for pair P; fine=True tiles the
                # moving operand at s-quarters so pair 0 starts as soon as
                # the first transpose quarter lands --
                if wq is None:
                    wq = fetch_wq(P)
                qT = qkpool.tile([128, 1024], bf16, tag="qT", name="qT")
                kT = qkpool.tile([128, 1024], bf16, tag="kT", name="kT")
                qT_t[P] = qT
                kT_t[P] = kT
                nq = 2 if fine else 1
                for dst, bcol, ct in ((qT, 0, P), (kT, 128, 8 + P)):
                    for half in range(2):
                        ps = qkps.tile([128, 512], f32, tag="qkp", name="qkp")
                        for qs in range(nq):
                            w = 512 // nq
                            for c in range(8):
                                nc.tensor.matmul(
                                    ps[:, w * qs:w * (qs + 1)],
                                    wq[:, 256 * c + bcol:256 * c + bcol + 128],
                                    hsT_ap(c, 512 * half + w * qs, w),
                                    start=(qs == 0 and c == 0),
                                    stop=(qs == nq - 1 and c == 7),
                                    skip_group_check=True)
                        nc.scalar.activation(
                            dst[:, 512 * half:512 * half + 512], ps[:], Ident,
                            bias=bqk_sb[:, ct:ct + 1], scale=1.0)
                qd = [dpool.tile([128, 9216], fp8, tag=f"qd{hh}",
                                 name=f"qd{hh}") for hh in range(2)]
                kd = [dpool.tile([128, 9216], fp8, tag=f"kd{hh}",
                                 name=f"kd{hh}") for hh in range(2)]
                qd_t[P] = qd
                kd_t[P] = kd

            def emit_3b_chunk(P, t, kind):
                # -- windows for pair P, block t, one side (hh-interleaved
                # matmuls land on row groups 0-1/2-3 for HW concurrency) --
                # k-side (kind=1) windows are stored pre-scaled by 0.125 so
                # the score-side fold is pr = (1 + s2)*exp(...), replacing
                # the former id8xs2 PE matmul-add (first-order exp identity;
                # |0.125*Wk| ~ 0.013 makes the quadratic term negligible).
                base = 896 - 128 * t
                if kind == 0:
                    src, tbl, dst_list = qT_t[P], t2r_sb, qd_t[P]
                else:
                    src, tbl, dst_list = kT_t[P], t2_sb, kd_t[P]
                scale = 1.0
                stg = [stgpool.tile([128, WIN], fp8, tag="stg", name="stg")
                       for _ in range(2)]
                for c3 in range(3):
                    for hh in range(2):
                        rs = slice(64 * hh, 64 * (hh + 1))
                        # rotate window PSUM through both the win and the
                        # (mostly idle here) qkp tags -> 4 effective slots
                        wpool, wtag = ((winps, "win") if win_rr[0] % 2 == 0
                                       else (qkps, "qkp"))
                        win_rr[0] += 1
                        wp = wpool.tile([128, 384], f32, tag=wtag, name="win")
                        nc.tensor.matmul(
                            wp[:], src[rs, 128 * t:128 * (t + 1)],
                            tbl[rs, base + 384 * c3:base + 384 * (c3 + 1)],
                            start=True, stop=True,
                            skip_group_check=True)
                        # copy emitted right after its matmul (on ACT or DVE,
                        # alternating) so the PSUM slot recycles one matmul
                        # earlier; PE-stream hh-adjacency is unaffected.
                        sl = slice(384 * c3, 384 * (c3 + 1))
                        cnt = copy_rr[0]
                        copy_rr[0] += 1
                        if cnt % 16 in act_slots:
                            if scale == 1.0:
                                nc.scalar.copy(stg[hh][:, sl], wp[:])
                            else:
                                nc.scalar.mul(stg[hh][:, sl], wp[:], scale)
                        else:
                            if scale == 1.0:
                                nc.vector.tensor_copy(stg[hh][:, sl], wp[:])
                            else:
                                nc.vector.tensor_scalar_mul(
                                    stg[hh][:, sl], wp[:], scale)
                for hh in range(2):
                    nc.sync.dma_start(
                        dst_list[hh][:].__replace__(
                            ap=[[9216, 128], [1, WIN]], offset=WIN * t),
                        stg[hh][:])

            def emit_skew(P):
                # -- merged skew reads for pair P --
                s1p = []
                s2p = []
                for hh in range(2):
                    s1 = skpool.tile([128, 8192], fp8, tag=f"s1h{hh}",
                                     name=f"s1h{hh}")
                    nc.sync.dma_start(
                        s1[:].__replace__(
                            ap=[[8192, 128], [1024, 8], [1, 1024]], offset=0),
                        qd_t[P][hh][:].__replace__(
                            ap=[[9215, 128], [WIN, 8], [1, 1024]], offset=127))
                    s1p.append(s1)
                    s2 = skpool.tile([128, 8192], fp8, tag=f"s2h{hh}",
                                     name=f"s2h{hh}")
                    nc.sync.dma_start(
                        s2[:].__replace__(
                            ap=[[8192, 128], [1024, 8], [1, 1024]], offset=0),
                        kd_t[P][hh][:].__replace__(
                            ap=[[9215, 128], [WIN, 8], [1, 1024]], offset=127))
                    s2p.append(s2)
                s1_t[P] = s1p
                s2_t[P] = s2p

            def emit_phase2_group(tau, half):
                # one v-hat half-tile (+ones col, +bias), from resident WV
                if True:
                    if True:
                        psv = qkps.tile([128, 512], f32, tag="qkp", name="vps")
                        sl = slice(512 * half, 512 * (half + 1))
                        nc.tensor.matmul(psv[:], ones_sb[:], bv_sb[:, sl],
                                         start=True, stop=False,
                                         skip_group_check=True)
                        for c in range(8):
                            nc.tensor.matmul(
                                psv[:],
                                hsT_ap(c, 128 * tau, 128),
                                wv_sb[:, 1024 * c + sl.start:
                                      1024 * c + sl.stop],
                                start=False, stop=(c == 7),
                                skip_group_check=True)
                        out_ap = vh[tau][:].__replace__(
                            ap=[[1040, 128], [65, 8], [1, 64]],
                            offset=65 * 8 * half)
                        in_ap = psv[:].__replace__(
                            ap=[[512, 128], [64, 8], [1, 64]], offset=0)
                        nc.scalar.copy(out_ap, in_ap)
                    if half == 1:
                        ones_ap = vh[tau][:].__replace__(
                            ap=[[1040, 128], [65, 16]], offset=64)
                        nc.scalar.copy(ones_ap, ones16[:])

            pc_t = {}
            ctx_pending = []

            def flush_ctx(n_keep):
                while len(ctx_pending) > n_keep:
                    pc_tile, vh_ap, pr, t = ctx_pending.pop(0)
                    nc.tensor.matmul(
                        pc_tile[:], vh_ap, pr[:],
                        start=(t == 0), stop=(t == NT - 1),
                        skip_group_check=True)

            sc_live = {}

            def emit_3d_qk(P, hh, t, lh):
                # qk matmul: 64-row stationary like the window matmuls it
                # sits next to, avoiding a PE row-tile reconfig flush
                # (~96ns); writes the full tile with start=True.
                qT = qT_t[P]
                kT = kT_t[P]
                rs = slice(64 * hh, 64 * (hh + 1))
                if t == 0 and lh == 0:
                    pc_t[(P, hh)] = [
                        ctxps.tile([65, 512], f32, tag="pc", name="pc")
                        for _ in range(2)]
                sc = scps.tile([128, 512], f32, tag="sc", name="sc")
                sc_live[(P, hh, t, lh)] = sc
                nc.tensor.matmul(
                    sc[:], kT[rs, 128 * t:128 * (t + 1)],
                    qT[rs, 512 * lh:512 * (lh + 1)],
                    start=True, stop=False,
                    skip_group_check=True)

            def emit_3d_rest(P, hh, t, lh):
                h = 2 * P + hh
                s1 = s1_t[P][hh]
                s2 = s2_t[P][hh]
                pc = pc_t[(P, hh)]
                sc = sc_live.pop((P, hh, t, lh))
                if True:
                    for Lh in range(4):
                        L = 4 * lh + Lh
                        nc.tensor.matmul(
                            sc[:, 128 * Lh:128 * (Lh + 1)],
                            s1[:, 1024 * L + 128 * t:
                               1024 * L + 128 * (t + 1)],
                            id8[:],
                            start=False, stop=False,
                            skip_group_check=True)
                    nc.tensor.matmul(
                        sc[:], id8[:],
                        s2[:, 1024 * t + 512 * lh:
                           1024 * t + 512 * (lh + 1)],
                        start=False, stop=True,
                        skip_group_check=True)
                    pr = prpool.tile([128, 512], bf16, tag="pr", name="pr")
                    nc.scalar.activation(pr[:], sc[:], Exp,
                                         bias=mhat_sb[:, t:t + 1],
                                         scale=0.125)
                    # ctx matmul deferred by 2 score-groups so PE never
                    # waits on the exp -> pr dependency.
                    ctx_pending.append(
                        (pc[lh], vh[t][:, 65 * h:65 * (h + 1)], pr, t))
                    flush_ctx(int(os.environ.get("K_FLUSH", "4")))

            def emit_head_tail(P, hh):
                h = 2 * P + hh
                flush_ctx(0)
                pc = pc_t.pop((P, hh))
                cs = cspool.tile([65, 1024], f32, tag="cs", name="cs")
                for lh in range(2):
                    nc.vector.tensor_copy(cs[:, 512 * lh:512 * (lh + 1)],
                                          pc[lh][:])
                # Batched output transposes: 4 L-blocks per PSUM tile from
                # the (idle here) qkp tag, one strided reciprocal per tile,
                # so PE never interlocks with the DVE drain per column.
                for g in range(2):
                    po = qkps.tile([128, 512], f32, tag="qkp", name="po")
                    for Lh in range(4):
                        L = 4 * g + Lh
                        nc.tensor.transpose(
                            po[:, 65 * Lh:65 * Lh + 65],
                            cs[:, 128 * L:128 * (L + 1)],
                            idf[0:65, 0:65])
                    rec = osmall.tile([128, 4], f32, tag="rec", name="rec")
                    nc.vector.reciprocal(
                        rec[:],
                        po[:].__replace__(ap=[[512, 128], [65, 4]],
                                          offset=64))
                    for Lh in range(4):
                        L = 4 * g + Lh
                        nc.vector.tensor_scalar_mul(
                            outacc[:, 1024 * L + 64 * h:
                                   1024 * L + 64 * (h + 1)],
                            po[:, 65 * Lh:65 * Lh + 64],
                            rec[:, Lh:Lh + 1])

            def emit_pair_cleanup(P):
                qT_t.pop(P)
                kT_t.pop(P)
                s1_t.pop(P)
                s2_t.pop(P)
                qd_t.pop(P)
                kd_t.pop(P)

            # Software-pipelined emission: pair P+1's projection and window
            # half-chunks interleave 1:1 with pair P's 16 score chunks.
            # DMACopy <-> DMATranspose transitions serialize the DMA
            # pipeline (xbar-mode workaround), so issue all plain copies
            # first, then all four transpose quarters: one transition total.
            wq0[0] = fetch_wq(0)
            for qs in (0, 1, 2, 3):
                emit_hsT(qs)
            nc.sync.dma_start(bqk_sb[:], bqk_d.ap())
            emit_3a(0, wq0[0], fine=True)
            emit_consts_rest()
            # pair-0 window chunks interleave 1:1 with the v-projection
            # groups so their PSUM->fp8 copy drain hides behind v matmuls
            for i in range(16):
                emit_3b_chunk(0, i // 2, i % 2)
                emit_phase2_group(i // 2, i % 2)
            emit_skew(0)
            for P in range(npair):
                if P + 1 < npair:
                    emit_3a(P + 1)
                for hh in range(2):
                    for t in range(NT):
                        i = hh * NT + t
                        # window chunks front-loaded into the first 8 score
                        # iterations (split around the two score groups so
                        # their PSUM copies drain while PE does score work);
                        # the skew round trip of pair P+1 then has the
                        # remaining 8 iterations of cover
                        if P + 1 < npair and i < NT:
                            emit_3b_chunk(P + 1, i, 0)
                            emit_3d_qk(P, hh, t, 0)
                            emit_3b_chunk(P + 1, i, 1)
                            emit_3d_qk(P, hh, t, 1)
                        else:
                            emit_3d_qk(P, hh, t, 0)
                            emit_3d_qk(P, hh, t, 1)
                        emit_3d_rest(P, hh, t, 0)
                        emit_3d_rest(P, hh, t, 1)
                    if hh == 0 and P + 1 < npair:
                        emit_skew(P + 1)
                    emit_head_tail(P, hh)
                    if P == 7 and hh == 0:
                        # head 14 final: stream its columns during head 15's
                        # score/ctx work to shorten the serial tail
                        nc.sync.dma_start(
                            out_d.ap().__replace__(
                                ap=[[1024, 128], [131072, 8], [1, 64]],
                                offset=896),
                            outacc[:].__replace__(
                                ap=[[8192, 128], [1024, 8], [1, 64]],
                                offset=896))
                emit_pair_cleanup(P)
                if P in (1, 3, 5):
                    # output columns for finished head quads are final:
                    # stream them while later pairs compute (one DMA)
                    c0 = 256 * ((P - 1) // 2)
                    nc.sync.dma_start(
                        out_d.ap().__replace__(
                            ap=[[1024, 128], [131072, 8], [1, 256]],
                            offset=c0),
                        outacc[:].__replace__(
                            ap=[[8192, 128], [1024, 8], [1, 256]],
                            offset=c0))
                elif P == 6:
                    # heads 12-13 final: stream cols 768-895 during pair 7
                    nc.sync.dma_start(
                        out_d.ap().__replace__(
                            ap=[[1024, 128], [131072, 8], [1, 128]],
                            offset=768),
                        outacc[:].__replace__(
                            ap=[[8192, 128], [1024, 8], [1, 128]],
                            offset=768))

            for g in range(2):
                nc.sync.dma_start(
                    out_d.ap().__replace__(
                        ap=[[1024, 128], [131072, 4], [1, 128]],
                        offset=896 + 4 * g * 131072),
                    outacc[:].__replace__(
                        ap=[[8192, 128], [1024, 4], [1, 128]],
                        offset=896 + 4 * g * 1024))

    nc.compile()
    return nc


_NC_CACHE = {}
_LAST = {"exec_time_ns": None, "results": None}


def _get_program():
    if "nc" not in _NC_CACHE:
        _NC_CACHE["nc"] = build_program()
    return _NC_CACHE["nc"]


def get_last_exec_time_ns():
    return _LAST["exec_time_ns"]


def get_last_results():
    return _LAST["results"]


def kernel(hidden_states, attention_mask, W_qkv, b_qkv, dist_emb):
    from concourse.bass_utils import run_bass_kernel_spmd

    hidden_states = np.asarray(hidden_states, dtype=np.float32)
    attention_mask = np.asarray(attention_mask, dtype=np.float32)
    W_qkv = np.asarray(W_qkv, dtype=np.float32)
    b_qkv = np.asarray(b_qkv, dtype=np.float32)
    dist_emb = np.asarray(dist_emb, dtype=np.float32)

    B = hidden_states.shape[0]
    nc = _get_program()
    in_maps = host_prep(hidden_states, attention_mask, W_qkv, b_qkv, dist_emb)
    trace = bool(os.environ.get("BASS_TRACE"))
    tracedir = os.environ.get("BASS_TRACE_DIR") or None
    try:
        res = run_bass_kernel_spmd(
            nc, in_maps, list(range(B)), trace=trace, tmpdir=tracedir)
    except ModuleNotFoundError:
        res = run_bass_kernel_spmd(nc, in_maps, list(range(B)), trace=False)
    _LAST["exec_time_ns"] = res.exec_time_ns
    _LAST["results"] = res
    out = np.stack([res.results[i]["out"] for i in range(B)], axis=0)
    return out.astype(np.float32)

